# revision 1
# baseline (speedup 1.0000x reference)
"""LSTMCell on 8 Trainium2 NeuronCores, data-parallel over the batch.

Full inputs: x/h_t/c_t [65536,128] f32, 8 gate weight matrices [128,128],
4 biases [128]. Returns (h_new, c_new) as [65536,128] f32 each.

Per core (8192 rows): batch tiles of 128 rows, groups of 4 tiles.
  - PE transposes x/h tiles (fp32) into PSUM, DVE copies them to SBUF
    rounding to f32r.
  - Two f32r matmuls per tile accumulate gates [128 batch, 512] into one
    PSUM bank; 4 tiles share a [128,2048] 4-bank "quad" tile.
  - Gate order [i, f, o, g] with W_g,b_g pre-scaled by 2 on host, so ONE
    sigmoid over the whole quad computes i,f,o and s=sigmoid(2g_a);
    tanh(g_a) = 2s-1 is a fused DVE tensor_scalar.
  - c_new = f*c + i*g on DVE; tanh(c_new) per group on ACT; h_new = o*tanh.
"""
import numpy as np
from contextlib import ExitStack

import concourse.bass as bass
import concourse.tile as tile
from concourse import bacc, mybir
from concourse.bass_utils import run_bass_kernel_spmd
from concourse.masks import make_identity

F32 = mybir.dt.float32
F32R = mybir.dt.float32r
AF = mybir.ActivationFunctionType
ALU = mybir.AluOpType

NCORES = 8
BC = 8192            # batch rows per core
GROUP_ROWS = 512     # 4 tiles of 128
NT = 4               # tiles per group
NG = BC // GROUP_ROWS

_CACHE = {}


def _build(has_bias: bool):
    nc = bacc.Bacc("TRN2", target_bir_lowering=False, debug=False)
    x = nc.dram_tensor("x", [BC, 128], F32, kind="ExternalInput").ap()
    h = nc.dram_tensor("h", [BC, 128], F32, kind="ExternalInput").ap()
    c = nc.dram_tensor("c", [BC, 128], F32, kind="ExternalInput").ap()
    wxt = nc.dram_tensor("wxt", [128, 512], F32R, kind="ExternalInput").ap()
    wht = nc.dram_tensor("wht", [128, 512], F32R, kind="ExternalInput").ap()
    if has_bias:
        bias = nc.dram_tensor("bias", [1, 512], F32R, kind="ExternalInput").ap()
    hn = nc.dram_tensor("hn", [BC, 128], F32, kind="ExternalOutput").ap()
    cn = nc.dram_tensor("cn", [BC, 128], F32, kind="ExternalOutput").ap()

    with tile.TileContext(nc) as tc:
        with ExitStack() as ctx:
            const = ctx.enter_context(tc.tile_pool(name="const", bufs=1))
            inp = ctx.enter_context(tc.tile_pool(name="inp", bufs=4))
            xht = ctx.enter_context(tc.tile_pool(name="xht", bufs=8))
            qp = ctx.enter_context(tc.tile_pool(name="qp", bufs=2, space="PSUM"))
            sp = ctx.enter_context(tc.tile_pool(name="sp", bufs=4))
            op = ctx.enter_context(tc.tile_pool(name="op", bufs=6))
            tmp = ctx.enter_context(tc.tile_pool(name="tmp", bufs=6))

            ident = const.tile([128, 128], F32)
            make_identity(nc, ident)
            wx_sb = const.tile([128, 512], F32R)
            nc.sync.dma_start(wx_sb[:], wxt)
            wh_sb = const.tile([128, 512], F32R)
            nc.sync.dma_start(wh_sb[:], wht)
            if has_bias:
                ones = const.tile([1, 128], F32R)
                nc.vector.memset(ones[:], 1.0)
                b_sb = const.tile([1, 512], F32R)
                nc.sync.dma_start(b_sb[:], bias)

            warm = qp.tile([128, 2048], F32, name="warm", tag="quad")
            for _ in range(16):
                nc.tensor.matmul(warm[:, 0:128], ident[:], ident[:],
                                 is_transpose=True, start=True, stop=True)

            xsl = hsl = csl = None
            for g in range(NG):
                r0 = g * GROUP_ROWS
                if g % 2 == 0:
                    # superload: 2 groups (1024 rows, 512KB) per dma_start
                    xsl = inp.tile([128, 2 * GROUP_ROWS], F32,
                                   name=f"xsl{g}", tag="xg")
                    hsl = inp.tile([128, 2 * GROUP_ROWS], F32,
                                   name=f"hsl{g}", tag="hg")
                    csl = inp.tile([128, 2 * GROUP_ROWS], F32,
                                   name=f"csl{g}", tag="cg")
                    for sb_t, dram in ((xsl, x), (hsl, h), (csl, c)):
                        nc.sync.dma_start(
                            sb_t[:].rearrange("p (t f) -> p t f", t=2 * NT),
                            dram[r0:r0 + 2 * GROUP_ROWS, :].rearrange(
                                "(t p) f -> p t f", p=128))
                off = (g % 2) * GROUP_ROWS
                xg = xsl[:, off:off + GROUP_ROWS]
                hg = hsl[:, off:off + GROUP_ROWS]
                cg = csl[:, off:off + GROUP_ROWS]

                quad = qp.tile([128, 2048], F32, name=f"quad{g}", tag="quad")
                # pass A: all transposes back-to-back on PE, then ONE wide
                # rounding cast over all 4 banks (strided 3D AP)
                for t in range(NT):
                    col = t * 512
                    fs = t * 128
                    nc.tensor.matmul(quad[:, col:col + 128],
                                     xg[:, fs:fs + 128], ident[:],
                                     is_transpose=True, start=True, stop=False)
                    nc.tensor.matmul(quad[:, col + 128:col + 256],
                                     hg[:, fs:fs + 128], ident[:],
                                     is_transpose=True, start=False, stop=True)
                xh_w = xht.tile([128, 1024], F32R, name=f"xh{g}", tag="xh")
                nc.vector.tensor_copy(
                    xh_w[:].rearrange("p (t x) -> p t x", t=NT),
                    quad[:].rearrange("p (t x) -> p t x", t=NT)[:, :, 0:256])
                # pass B: all gates matmuls = [x h] @ [WxT; WhT] (+ bias)
                for t in range(NT):
                    col = t * 512
                    xh = xh_w[:, t * 256:(t + 1) * 256]
                    first = True
                    if has_bias:
                        nc.tensor.matmul(quad[:, col:col + 512], ones[:],
                                         b_sb[:], start=True, stop=False)
                        first = False
                    nc.tensor.matmul(quad[:, col:col + 512], xh[:, 0:128],
                                     wx_sb[:], start=first, stop=False)
                    nc.tensor.matmul(quad[:, col:col + 512], xh[:, 128:256],
                                     wh_sb[:], start=False, stop=True)

                sig = sp.tile([128, 2048], F32, name=f"sig{g}", tag="sig")
                cn_g = op.tile([128, GROUP_ROWS], F32, name=f"cn{g}", tag="cn")
                tc_g = op.tile([128, GROUP_ROWS], F32, name=f"tc{g}", tag="tcg")
                hn_g = op.tile([128, GROUP_ROWS], F32, name=f"hn{g}", tag="hn")
                # one sigmoid over the whole quad (4 banks), then wide DVE
                # ops via 3D (p, t, x) access patterns over all 4 tiles.
                nc.scalar.activation(sig[:], quad[:], AF.Sigmoid)
                sig3 = sig[:].rearrange("p (t x) -> p t x", t=NT)
                i_ap = sig3[:, :, 0:128]
                f_ap = sig3[:, :, 128:256]
                o_ap = sig3[:, :, 256:384]
                s_ap = sig3[:, :, 384:512]
                c3 = cg[:].rearrange("p (t x) -> p t x", t=NT)
                gt = tmp.tile([128, 512], F32, name=f"gt{g}", tag="gt")
                gt3 = gt[:].rearrange("p (t x) -> p t x", t=NT)
                nc.vector.tensor_scalar(gt3, s_ap, 2.0, 1.0,
                                        ALU.mult, ALU.subtract)
                ig = tmp.tile([128, 512], F32, name=f"ig{g}", tag="ig")
                ig3 = ig[:].rearrange("p (t x) -> p t x", t=NT)
                nc.vector.tensor_mul(ig3, i_ap, gt3)
                fc = tmp.tile([128, 512], F32, name=f"fc{g}", tag="fc")
                fc3 = fc[:].rearrange("p (t x) -> p t x", t=NT)
                nc.vector.tensor_mul(fc3, f_ap, c3)
                nc.vector.tensor_add(cn_g[:], ig[:], fc[:])
                nc.scalar.activation(tc_g[:], cn_g[:], AF.Tanh)
                tc3 = tc_g[:].rearrange("p (t x) -> p t x", t=NT)
                hn3 = hn_g[:].rearrange("p (t x) -> p t x", t=NT)
                nc.vector.tensor_mul(hn3, o_ap, tc3)
                for sb_t, dram in ((hn_g, hn), (cn_g, cn)):
                    nc.sync.dma_start(
                        dram[r0:r0 + GROUP_ROWS, :].rearrange(
                            "(t p) f -> p t f", p=128),
                        sb_t[:].rearrange("p (t f) -> p t f", t=NT))
    nc.compile()
    return nc


def _run(inputs, trace=False, tmpdir=None):
    x = np.ascontiguousarray(inputs["x"], dtype=np.float32)
    h = np.ascontiguousarray(inputs["h_t"], dtype=np.float32)
    c = np.ascontiguousarray(inputs["c_t"], dtype=np.float32)
    # gate order [i, f, o, g]; W_g/b_g scaled by 2 for the tanh-via-sigmoid
    wx = np.concatenate([inputs["W_ii"], inputs["W_if"], inputs["W_io"],
                         2.0 * np.asarray(inputs["W_ig"])], axis=0)
    wh = np.concatenate([inputs["W_hi"], inputs["W_hf"], inputs["W_ho"],
                         2.0 * np.asarray(inputs["W_hg"])], axis=0)
    b = np.concatenate([inputs["b_i"], inputs["b_f"], inputs["b_o"],
                        2.0 * np.asarray(inputs["b_g"])], axis=0)
    wxt = np.ascontiguousarray(wx.T, dtype=np.float32)
    wht = np.ascontiguousarray(wh.T, dtype=np.float32)
    has_bias = bool(np.any(b))

    key = has_bias
    if key not in _CACHE:
        _CACHE[key] = _build(has_bias)
    nc = _CACHE[key]

    in_maps = []
    for i in range(NCORES):
        m = {
            "x": x[i * BC:(i + 1) * BC],
            "h": h[i * BC:(i + 1) * BC],
            "c": c[i * BC:(i + 1) * BC],
            "wxt": wxt,
            "wht": wht,
        }
        if has_bias:
            m["bias"] = b.reshape(1, 512).astype(np.float32)
        in_maps.append(m)

    res = run_bass_kernel_spmd(nc, in_maps, core_ids=list(range(NCORES)),
                               trace=trace, tmpdir=tmpdir)
    h_new = np.concatenate([r["hn"] for r in res.results], axis=0)
    c_new = np.concatenate([r["cn"] for r in res.results], axis=0)
    return h_new, c_new, res


def kernel(**inputs):
    h_new, c_new, _ = _run(inputs, trace=False)
    return h_new, c_new



# revision 2
# speedup vs baseline: 1.0897x; 1.0897x over previous
"""LSTMCell on 8 Trainium2 NeuronCores, data-parallel over the batch.

Full inputs: x/h_t/c_t [65536,128] f32, 8 gate weight matrices [128,128],
4 biases [128]. Returns (h_new, c_new) as [65536,128] f32 each.

v2 design (fp16 I/O, transposed layout, zero on-device transposes):
  - Host transposes x/h/c per core to xT/hT/cT [128 feat, 8192 batch] fp16
    and pre-concats weights as WxT/WhT [128 in, 512 gate-rows] fp16 in gate
    order [i, f, o, 2*g] (g prescaled by 2 for the tanh-via-sigmoid trick).
  - Per batch group of 512 columns: 8 fp16 matmuls (weights stationary,
    batch streams) accumulate gates^T into one 4-bank PSUM quad
    [128 gate, 2048]: bank0=i, bank1=f, bank2=o, bank3=2g pre-acts.
  - ONE sigmoid over the whole quad -> SBUF fp16 (i, f, o, s=sig(2g));
    tanh(g_a) = 2s-1 folded into the DVE chain.
  - DVE (all fp16 SBUF->SBUF, 2x mode): t1=(s-0.5)*i [STT], fc=f*c [TT],
    c' = 2*t1 + fc [STT], h' = o*tanh(c') [TT].
  - tanh(c') on ACT per pair of groups; outputs DMA'd out fp16 per 4 groups.
  - Host converts outputs back to f32 [65536,128].
"""
import numpy as np
from contextlib import ExitStack

import concourse.bass as bass
import concourse.tile as tile
from concourse import bacc, mybir
from concourse.bass_utils import run_bass_kernel_spmd

F32 = mybir.dt.float32
F16 = mybir.dt.float16
NPDT = np.float16
AF = mybir.ActivationFunctionType
ALU = mybir.AluOpType

NCORES = 8
BC = 8192            # batch rows per core
GW = 512             # batch columns per group (one PSUM bank)
NG = BC // GW        # 16 groups
CHUNK = 4            # groups per DMA chunk
H = 128              # hidden size

_CACHE = {}


def _build(has_bias: bool):
    nc = bacc.Bacc("TRN2", target_bir_lowering=False, debug=False)
    xt = nc.dram_tensor("xt", [H, BC], F16, kind="ExternalInput").ap()
    ht = nc.dram_tensor("ht", [H, BC], F16, kind="ExternalInput").ap()
    ct = nc.dram_tensor("ct", [H, BC], F16, kind="ExternalInput").ap()
    wxt = nc.dram_tensor("wxt", [H, 4 * H], F16, kind="ExternalInput").ap()
    wht = nc.dram_tensor("wht", [H, 4 * H], F16, kind="ExternalInput").ap()
    if has_bias:
        bias = nc.dram_tensor("bias", [H, 4], F32, kind="ExternalInput").ap()
    hnt = nc.dram_tensor("hnt", [H, BC], F16, kind="ExternalOutput").ap()
    cnt = nc.dram_tensor("cnt", [H, BC], F16, kind="ExternalOutput").ap()

    CW = CHUNK * GW  # 2048 columns per DMA chunk

    with tile.TileContext(nc) as tc:
        with ExitStack() as ctx:
            const = ctx.enter_context(tc.tile_pool(name="const", bufs=1))
            inp = ctx.enter_context(tc.tile_pool(name="inp", bufs=2))
            qp = ctx.enter_context(tc.tile_pool(name="qp", bufs=2, space="PSUM"))
            sp = ctx.enter_context(tc.tile_pool(name="sp", bufs=3))
            tp = ctx.enter_context(tc.tile_pool(name="tp", bufs=3))
            op = ctx.enter_context(tc.tile_pool(name="op", bufs=2))

            wx_sb = const.tile([H, 4 * H], F16)
            nc.sync.dma_start(wx_sb[:], wxt)
            wh_sb = const.tile([H, 4 * H], F16)
            nc.sync.dma_start(wh_sb[:], wht)
            if has_bias:
                b_sb = const.tile([H, 4], F32)
                nc.sync.dma_start(b_sb[:], bias)

            # ACT table preload (sigmoid_and_others, includes tanh) at t=0
            # so the ~2.7us table DMA overlaps the input DMAs.
            dummy = const.tile([H, 8], F32)
            nc.vector.memset(dummy[:], 0.0)
            dummy2 = const.tile([H, 8], F32)
            nc.scalar.activation(dummy2[:], dummy[:], AF.Sigmoid)

            # HAM warmup: real (non-transpose) matmuls on a memset junk tile
            # keep the PE activity window busy while input DMAs stream.
            junk = const.tile([H, GW], F16)
            nc.vector.memset(junk[:], 0.0)
            warm = qp.tile([H, 2048], F32, name="warm", tag="quad")
            for _ in range(8):
                nc.tensor.matmul(warm[:, 0:GW], junk[:, 0:H], junk[:],
                                 start=True, stop=True)

            xc = hc = cc = None
            cn_buf = hn_buf = tc_buf = None
            col0 = 0
            prev_o = None
            for g in range(NG):
                ci, gl = divmod(g, CHUNK)
                if gl == 0:
                    col0 = ci * CW
                    xc = inp.tile([H, CW], F16, name=f"x{ci}", tag="xg")
                    hc = inp.tile([H, CW], F16, name=f"h{ci}", tag="hg")
                    cc = inp.tile([H, CW], F16, name=f"c{ci}", tag="cg")
                    nc.sync.dma_start(xc[:], xt[:, col0:col0 + CW])
                    nc.sync.dma_start(hc[:], ht[:, col0:col0 + CW])
                    nc.sync.dma_start(cc[:], ct[:, col0:col0 + CW])
                    cn_buf = op.tile([H, CW], F16, name=f"cn{ci}", tag="cn")
                    hn_buf = op.tile([H, CW], F16, name=f"hn{ci}", tag="hn")
                if gl % 2 == 0:
                    tc_buf = tp.tile([H, 2 * GW], F16, name=f"tc{g}", tag="tc")

                quad = qp.tile([H, 2048], F32, name=f"q{g}", tag="quad")
                xs = xc[:, gl * GW:(gl + 1) * GW]
                hs = hc[:, gl * GW:(gl + 1) * GW]
                for k in range(4):
                    nc.tensor.matmul(quad[:, k * GW:(k + 1) * GW],
                                     wx_sb[:, k * H:(k + 1) * H], xs,
                                     start=True, stop=False)
                    nc.tensor.matmul(quad[:, k * GW:(k + 1) * GW],
                                     wh_sb[:, k * H:(k + 1) * H], hs,
                                     start=False, stop=True)

                sig = sp.tile([H, 2048], F16, name=f"s{g}", tag="sig")
                if has_bias:
                    for k in range(4):
                        nc.scalar.activation(sig[:, k * GW:(k + 1) * GW],
                                             quad[:, k * GW:(k + 1) * GW],
                                             AF.Sigmoid, bias=b_sb[:, k:k + 1])
                else:
                    nc.scalar.activation(sig[:], quad[:], AF.Sigmoid)
                i_ap = sig[:, 0:GW]
                f_ap = sig[:, GW:2 * GW]
                o_ap = sig[:, 2 * GW:3 * GW]
                s_ap = sig[:, 3 * GW:4 * GW]

                t1 = tp.tile([H, GW], F16, name=f"t1{g}", tag="t1")
                nc.vector.scalar_tensor_tensor(t1[:], s_ap, 0.5, i_ap,
                                               ALU.subtract, ALU.mult)
                fc = tp.tile([H, GW], F16, name=f"fc{g}", tag="fc")
                nc.vector.tensor_mul(fc[:], f_ap, cc[:, gl * GW:(gl + 1) * GW])
                cns = cn_buf[:, gl * GW:(gl + 1) * GW]
                nc.vector.scalar_tensor_tensor(cns, t1[:], 2.0, fc[:],
                                               ALU.mult, ALU.add)
                if gl % 2 == 1:
                    p0 = (gl - 1) * GW
                    if gl == CHUNK - 1:
                        # c' chunk complete: ship it while tanh/h' run
                        nc.sync.dma_start(cnt[:, col0:col0 + CW], cn_buf[:])
                    nc.scalar.activation(tc_buf[:], cn_buf[:, p0:p0 + 2 * GW],
                                         AF.Tanh)
                    nc.vector.tensor_mul(hn_buf[:, p0:p0 + GW], prev_o,
                                         tc_buf[:, 0:GW])
                    nc.vector.tensor_mul(hn_buf[:, p0 + GW:p0 + 2 * GW], o_ap,
                                         tc_buf[:, GW:2 * GW])
                    if gl == CHUNK - 1:
                        nc.sync.dma_start(hnt[:, col0:col0 + CW], hn_buf[:])
                prev_o = o_ap
    nc.compile()
    return nc


def _run(inputs, trace=False, tmpdir=None):
    x = np.asarray(inputs["x"], dtype=np.float32)
    h = np.asarray(inputs["h_t"], dtype=np.float32)
    c = np.asarray(inputs["c_t"], dtype=np.float32)
    # gate order [i, f, o, g]; W_g/b_g scaled by 2 for the tanh-via-sigmoid
    wx = np.concatenate([inputs["W_ii"], inputs["W_if"], inputs["W_io"],
                         2.0 * np.asarray(inputs["W_ig"])], axis=0)
    wh = np.concatenate([inputs["W_hi"], inputs["W_hf"], inputs["W_ho"],
                         2.0 * np.asarray(inputs["W_hg"])], axis=0)
    b = np.concatenate([inputs["b_i"], inputs["b_f"], inputs["b_o"],
                        2.0 * np.asarray(inputs["b_g"])], axis=0)
    wxt = np.ascontiguousarray(wx.T, dtype=NPDT)
    wht = np.ascontiguousarray(wh.T, dtype=NPDT)
    has_bias = bool(np.any(b))

    key = has_bias
    if key not in _CACHE:
        _CACHE[key] = _build(has_bias)
    nc = _CACHE[key]

    x16 = x.astype(NPDT)
    h16 = h.astype(NPDT)
    c16 = c.astype(NPDT)
    in_maps = []
    for i in range(NCORES):
        s = slice(i * BC, (i + 1) * BC)
        m = {
            "xt": np.ascontiguousarray(x16[s].T),
            "ht": np.ascontiguousarray(h16[s].T),
            "ct": np.ascontiguousarray(c16[s].T),
            "wxt": wxt,
            "wht": wht,
        }
        if has_bias:
            m["bias"] = np.ascontiguousarray(
                b.reshape(4, H).T.astype(np.float32))
        in_maps.append(m)

    res = run_bass_kernel_spmd(nc, in_maps, core_ids=list(range(NCORES)),
                               trace=trace, tmpdir=tmpdir)
    h_new = np.empty((NCORES * BC, H), dtype=np.float32)
    c_new = np.empty((NCORES * BC, H), dtype=np.float32)
    for i, r in enumerate(res.results):
        s = slice(i * BC, (i + 1) * BC)
        h_new[s] = r["hnt"].T
        c_new[s] = r["cnt"].T
    return h_new, c_new, res


def kernel(**inputs):
    h_new, c_new, _ = _run(inputs, trace=False)
    return h_new, c_new


# revision 3
# speedup vs baseline: 1.3387x; 1.2285x over previous
"""LSTMCell on 8 Trainium2 NeuronCores, data-parallel over the batch.

Full inputs: x/h_t/c_t [65536,128] f32, 8 gate weight matrices [128,128],
4 biases [128]. Returns (h_new, c_new) as [65536,128] f32 each.

v3 design (all-bf16, transposed layout, zero on-device transposes):
  - Host transposes x/h/c per core to xT/hT/cT [128 feat, 8192 batch] bf16
    and pre-concats weights as WxT/WhT [128 in, 512 gate-rows] bf16 in gate
    order [i, f, o, 2*g] (g prescaled by 2 for the tanh-via-sigmoid trick).
  - Per batch group of 512 columns: 8 bf16 matmuls (weights stationary,
    batch streams; PE issue period ~259ns) accumulate gates^T into a 4-bank
    PSUM quad [128, 2048]: banks = i|f|o|2g pre-acts.
  - ONE sigmoid per quad -> bf16 SBUF (bf16 ACT output is ~1.7x faster than
    fp16 out); two groups share a sig2 tile [128, 4096] so DVE ops batch
    per PAIR of groups via 3D APs (keeps the DVE 2x bf16 mode).
  - DVE per pair: gt=2s-1 [TS 4x], ig=i*gt [TT], fc=f*c [TT],
    c'=ig+fc [TT], h'=o*tanh(c') [TT].
  - tanh(c') on ACT, delayed one pair so ACT never stalls on the DVE chain.
  - DMA: inputs split 4+12 groups (fast fill + big descriptors), outputs
    per 8 groups (1MB, 8KB/partition descriptors).
"""
import numpy as np
import ml_dtypes
from contextlib import ExitStack

import concourse.bass as bass
import concourse.tile as tile
from concourse import bacc, mybir
from concourse.bass_utils import run_bass_kernel_spmd

F32 = mybir.dt.float32
BF16 = mybir.dt.bfloat16
NPDT = ml_dtypes.bfloat16
AF = mybir.ActivationFunctionType
ALU = mybir.AluOpType

NCORES = 8
BC = 8192            # batch rows per core
GW = 512             # batch columns per group (one PSUM bank)
NG = BC // GW        # 16 groups
H = 128              # hidden size
SPLIT = 4            # groups in the first (small) input chunk
OCH = 8              # groups per output DMA chunk

_CACHE = {}


def _build(has_bias: bool):
    nc = bacc.Bacc("TRN2", target_bir_lowering=False, debug=False)
    xt = nc.dram_tensor("xt", [H, BC], BF16, kind="ExternalInput").ap()
    ht = nc.dram_tensor("ht", [H, BC], BF16, kind="ExternalInput").ap()
    ct = nc.dram_tensor("ct", [H, BC], BF16, kind="ExternalInput").ap()
    wxt = nc.dram_tensor("wxt", [H, 4 * H], BF16, kind="ExternalInput").ap()
    wht = nc.dram_tensor("wht", [H, 4 * H], BF16, kind="ExternalInput").ap()
    if has_bias:
        bias = nc.dram_tensor("bias", [H, 4], F32, kind="ExternalInput").ap()
    hnt = nc.dram_tensor("hnt", [H, BC], BF16, kind="ExternalOutput").ap()
    cnt = nc.dram_tensor("cnt", [H, BC], BF16, kind="ExternalOutput").ap()

    SW = SPLIT * GW          # small-chunk width (cols)
    BW = (NG - SPLIT) * GW   # big-chunk width
    OW = OCH * GW            # output chunk width

    with tile.TileContext(nc) as tc:
        with ExitStack() as ctx:
            const = ctx.enter_context(tc.tile_pool(name="const", bufs=1))
            ina = ctx.enter_context(tc.tile_pool(name="ina", bufs=1))
            qp = ctx.enter_context(tc.tile_pool(name="qp", bufs=2, space="PSUM"))
            sp = ctx.enter_context(tc.tile_pool(name="sp", bufs=3))
            tp = ctx.enter_context(tc.tile_pool(name="tp", bufs=2))
            op = ctx.enter_context(tc.tile_pool(name="op", bufs=2))

            wx_sb = const.tile([H, 4 * H], BF16)
            nc.sync.dma_start(wx_sb[:], wxt)
            wh_sb = const.tile([H, 4 * H], BF16)
            nc.sync.dma_start(wh_sb[:], wht)
            if has_bias:
                b_sb = const.tile([H, 4], F32)
                nc.sync.dma_start(b_sb[:], bias)

            # ACT table preload (sigmoid set includes tanh) overlaps DMA fill
            dummy = const.tile([H, 8], F32)
            nc.vector.memset(dummy[:], 0.0)
            dummy2 = const.tile([H, 8], F32)
            nc.scalar.activation(dummy2[:], dummy[:], AF.Sigmoid)

            # input tiles: small chunk (fast pipeline fill) + big chunk
            xa = ina.tile([H, SW], BF16, name="xa")
            ha = ina.tile([H, SW], BF16, name="ha")
            ca = ina.tile([H, SW], BF16, name="ca")
            xb = ina.tile([H, BW], BF16, name="xb")
            hb = ina.tile([H, BW], BF16, name="hb")
            cb = ina.tile([H, BW], BF16, name="cb")
            nc.sync.dma_start(xa[:], xt[:, 0:SW])
            nc.sync.dma_start(ha[:], ht[:, 0:SW])
            nc.sync.dma_start(ca[:], ct[:, 0:SW])
            nc.sync.dma_start(xb[:], xt[:, SW:BC])
            nc.sync.dma_start(hb[:], ht[:, SW:BC])
            nc.sync.dma_start(cb[:], ct[:, SW:BC])

            def in_slice(t_small, t_big, g, w):
                c0 = g * GW
                if c0 + w <= SW:
                    return t_small[:, c0:c0 + w]
                return t_big[:, c0 - SW:c0 - SW + w]

            # HAM warmup on a junk tile while DMAs stream
            junk = const.tile([H, GW], BF16)
            nc.vector.memset(junk[:], 0.0)
            warm = qp.tile([H, 2048], F32, name="warm", tag="quad")
            for _ in range(8):
                nc.tensor.matmul(warm[:, 0:GW], junk[:, 0:H], junk[:],
                                 start=True, stop=True)

            NP = NG // 2  # pairs
            sig2s = {}
            tcs = {}
            cn_buf = hn_buf = None
            ocol = 0

            def emit_tanh_h(P):
                """tanh + h' + (maybe) hn DMA for pair P (c' already done)."""
                pl = (2 * P) % OCH  # first group of pair, local to out chunk
                cnb, hnb = cn_hn[P]
                tcp = tp.tile([H, 1024], BF16, name=f"tc{P}", tag="tc")
                nc.scalar.activation(tcp[:], cnb[:, pl * GW:(pl + 2) * GW],
                                     AF.Tanh)
                sig2 = sig2s.pop(P)
                o3 = sig2[:].rearrange("p (t x) -> p t x", t=2)[:, :, 1024:1536]
                h3 = hnb[:, pl * GW:(pl + 2) * GW].rearrange(
                    "p (t x) -> p t x", t=2)
                t3 = tcp[:].rearrange("p (t x) -> p t x", t=2)
                nc.vector.tensor_mul(h3, o3, t3)
                if pl == OCH - 2:
                    oc0 = (2 * P + 2 - OCH) * GW
                    nc.sync.dma_start(hnt[:, oc0:oc0 + OW], hnb[:])

            cn_hn = {}
            for P in range(NP):
                g0 = 2 * P
                if g0 % OCH == 0:
                    cn_buf = op.tile([H, OW], BF16, name=f"cn{g0}", tag="cn")
                    hn_buf = op.tile([H, OW], BF16, name=f"hn{g0}", tag="hn")
                    ocol = g0 * GW
                cn_hn[P] = (cn_buf, hn_buf)
                sig2 = sp.tile([H, 4096], BF16, name=f"s{P}", tag="sig")
                sig2s[P] = sig2

                for gg in range(2):
                    g = g0 + gg
                    xs = in_slice(xa, xb, g, GW)
                    hs = in_slice(ha, hb, g, GW)
                    quad = qp.tile([H, 2048], F32, name=f"q{g}", tag="quad")
                    for k in range(4):
                        nc.tensor.matmul(quad[:, k * GW:(k + 1) * GW],
                                         wx_sb[:, k * H:(k + 1) * H], xs,
                                         start=True, stop=False)
                        nc.tensor.matmul(quad[:, k * GW:(k + 1) * GW],
                                         wh_sb[:, k * H:(k + 1) * H], hs,
                                         start=False, stop=True)
                    so = sig2[:, gg * 2048:(gg + 1) * 2048]
                    if has_bias:
                        for k in range(4):
                            nc.scalar.activation(
                                so[:, k * GW:(k + 1) * GW],
                                quad[:, k * GW:(k + 1) * GW],
                                AF.Sigmoid, bias=b_sb[:, k:k + 1])
                    else:
                        nc.scalar.activation(so, quad[:], AF.Sigmoid)
                    if gg == 1 and P >= 1:
                        emit_tanh_h(P - 1)

                # DVE chain for pair P (3D APs batch both groups, 2x mode)
                s3 = sig2[:].rearrange("p (t x) -> p t x", t=2)
                i3 = s3[:, :, 0:512]
                f3 = s3[:, :, 512:1024]
                sg3 = s3[:, :, 1536:2048]
                cs = in_slice(ca, cb, g0, 1024)
                c3 = cs.rearrange("p (t x) -> p t x", t=2)
                gt = tp.tile([H, 1024], BF16, name=f"gt{P}", tag="gt")
                gt3 = gt[:].rearrange("p (t x) -> p t x", t=2)
                nc.vector.tensor_scalar(gt3, sg3, 2.0, 1.0,
                                        ALU.mult, ALU.subtract)
                ig = tp.tile([H, 1024], BF16, name=f"ig{P}", tag="ig")
                ig3 = ig[:].rearrange("p (t x) -> p t x", t=2)
                nc.vector.tensor_mul(ig3, i3, gt3)
                fc = tp.tile([H, 1024], BF16, name=f"fc{P}", tag="fc")
                fc3 = fc[:].rearrange("p (t x) -> p t x", t=2)
                nc.vector.tensor_mul(fc3, f3, c3)
                pl = g0 % OCH
                cns = cn_buf[:, pl * GW:(pl + 2) * GW]
                nc.vector.tensor_add(cns, ig[:], fc[:])
                if pl == OCH - 2:
                    nc.sync.dma_start(cnt[:, ocol:ocol + OW], cn_buf[:])

            emit_tanh_h(NP - 1)
    nc.compile()
    return nc


def _run(inputs, trace=False, tmpdir=None):
    x = np.asarray(inputs["x"], dtype=np.float32)
    h = np.asarray(inputs["h_t"], dtype=np.float32)
    c = np.asarray(inputs["c_t"], dtype=np.float32)
    # gate order [i, f, o, g]; W_g/b_g scaled by 2 for the tanh-via-sigmoid
    wx = np.concatenate([inputs["W_ii"], inputs["W_if"], inputs["W_io"],
                         2.0 * np.asarray(inputs["W_ig"])], axis=0)
    wh = np.concatenate([inputs["W_hi"], inputs["W_hf"], inputs["W_ho"],
                         2.0 * np.asarray(inputs["W_hg"])], axis=0)
    b = np.concatenate([inputs["b_i"], inputs["b_f"], inputs["b_o"],
                        2.0 * np.asarray(inputs["b_g"])], axis=0)
    wxt = np.ascontiguousarray(wx.T).astype(NPDT)
    wht = np.ascontiguousarray(wh.T).astype(NPDT)
    has_bias = bool(np.any(b))

    key = has_bias
    if key not in _CACHE:
        _CACHE[key] = _build(has_bias)
    nc = _CACHE[key]

    x16 = x.astype(NPDT)
    h16 = h.astype(NPDT)
    c16 = c.astype(NPDT)
    in_maps = []
    for i in range(NCORES):
        s = slice(i * BC, (i + 1) * BC)
        m = {
            "xt": np.ascontiguousarray(x16[s].T),
            "ht": np.ascontiguousarray(h16[s].T),
            "ct": np.ascontiguousarray(c16[s].T),
            "wxt": wxt,
            "wht": wht,
        }
        if has_bias:
            m["bias"] = np.ascontiguousarray(
                b.reshape(4, H).T.astype(np.float32))
        in_maps.append(m)

    res = run_bass_kernel_spmd(nc, in_maps, core_ids=list(range(NCORES)),
                               trace=trace, tmpdir=tmpdir)
    h_new = np.empty((NCORES * BC, H), dtype=np.float32)
    c_new = np.empty((NCORES * BC, H), dtype=np.float32)
    for i, r in enumerate(res.results):
        s = slice(i * BC, (i + 1) * BC)
        h_new[s] = r["hnt"].T
        c_new[s] = r["cnt"].T
    return h_new, c_new, res


def kernel(**inputs):
    h_new, c_new, _ = _run(inputs, trace=False)
    return h_new, c_new


# revision 10
# speedup vs baseline: 1.3438x; 1.0038x over previous
"""LSTMCell on 8 Trainium2 NeuronCores, data-parallel over the batch.

Full inputs: x/h_t/c_t [65536,128] f32, 8 gate weight matrices [128,128],
4 biases [128]. Returns (h_new, c_new) as [65536,128] f32 each.

v3 design (all-bf16, transposed layout, zero on-device transposes):
  - Host transposes x/h/c per core to xT/hT/cT [128 feat, 8192 batch] bf16
    and pre-concats weights as WxT/WhT [128 in, 512 gate-rows] bf16 in gate
    order [i, f, o, 2*g] (g prescaled by 2 for the tanh-via-sigmoid trick).
  - Per batch group of 512 columns: 8 bf16 matmuls (weights stationary,
    batch streams; PE issue period ~259ns) accumulate gates^T into a 4-bank
    PSUM quad [128, 2048]: banks = i|f|o|2g pre-acts.
  - ONE sigmoid per quad -> bf16 SBUF (bf16 ACT output is ~1.7x faster than
    fp16 out); two groups share a sig2 tile [128, 4096] so DVE ops batch
    per PAIR of groups via 3D APs (keeps the DVE 2x bf16 mode).
  - DVE per pair: gt=2s-1 [TS 4x], ig=i*gt [TT], fc=f*c [TT],
    c'=ig+fc [TT], h'=o*tanh(c') [TT].
  - tanh(c') on ACT, delayed one pair so ACT never stalls on the DVE chain.
  - DMA: inputs split 4+12 groups (fast fill + big descriptors), outputs
    per 8 groups (1MB, 8KB/partition descriptors).
"""
import numpy as np
import ml_dtypes
from contextlib import ExitStack

import concourse.bass as bass
import concourse.tile as tile
from concourse import bacc, mybir
from concourse.bass_utils import run_bass_kernel_spmd

F32 = mybir.dt.float32
BF16 = mybir.dt.bfloat16
NPDT = ml_dtypes.bfloat16
AF = mybir.ActivationFunctionType
ALU = mybir.AluOpType

NCORES = 8
BC = 8192            # batch rows per core
GW = 512             # batch columns per group (one PSUM bank)
NG = BC // GW        # 16 groups
H = 128              # hidden size
SPLIT = 2            # groups in the first (small) input chunk
# output chunks (start group, n groups): big, medium, small tail
OCHUNKS = [(0, 8), (8, 6), (14, 2)]

_CACHE = {}


def _build(has_bias: bool):
    nc = bacc.Bacc("TRN2", target_bir_lowering=False, debug=False)
    xt = nc.dram_tensor("xt", [H, BC], BF16, kind="ExternalInput").ap()
    ht = nc.dram_tensor("ht", [H, BC], BF16, kind="ExternalInput").ap()
    ct = nc.dram_tensor("ct", [H, BC], BF16, kind="ExternalInput").ap()
    clt = nc.dram_tensor("clt", [H, BC], BF16, kind="ExternalInput").ap()
    wxt = nc.dram_tensor("wxt", [H, 4 * H], BF16, kind="ExternalInput").ap()
    wht = nc.dram_tensor("wht", [H, 4 * H], BF16, kind="ExternalInput").ap()
    if has_bias:
        bias = nc.dram_tensor("bias", [H, 4], F32, kind="ExternalInput").ap()
    hnt = nc.dram_tensor("hnt", [H, BC], BF16, kind="ExternalOutput").ap()
    cnt = nc.dram_tensor("cnt", [H, BC], BF16, kind="ExternalOutput").ap()

    SW = SPLIT * GW          # small-chunk width (cols)
    BW = (NG - SPLIT) * GW   # big-chunk width

    with tile.TileContext(nc) as tc:
        with ExitStack() as ctx:
            const = ctx.enter_context(tc.tile_pool(name="const", bufs=1))
            ina = ctx.enter_context(tc.tile_pool(name="ina", bufs=1))
            qp = ctx.enter_context(tc.tile_pool(name="qp", bufs=2, space="PSUM"))
            sp = ctx.enter_context(tc.tile_pool(name="sp", bufs=3))
            tp = ctx.enter_context(tc.tile_pool(name="tp", bufs=2))
            op = ctx.enter_context(tc.tile_pool(name="op", bufs=2))

            # input tiles: small chunk (fast pipeline fill) + big chunk.
            # x/h first: they gate the first matmuls; weights are tiny.
            xa = ina.tile([H, SW], BF16, name="xa")
            ha = ina.tile([H, SW], BF16, name="ha")
            ca = ina.tile([H, SW], BF16, name="ca")
            cla = ina.tile([H, SW], BF16, name="cla")
            xb = ina.tile([H, BW], BF16, name="xb")
            hb = ina.tile([H, BW], BF16, name="hb")
            cb = ina.tile([H, BW], BF16, name="cb")
            clb = ina.tile([H, BW], BF16, name="clb")
            nc.sync.dma_start(xa[:], xt[:, 0:SW])
            nc.sync.dma_start(ha[:], ht[:, 0:SW])
            wx_sb = const.tile([H, 4 * H], BF16)
            nc.sync.dma_start(wx_sb[:], wxt)
            wh_sb = const.tile([H, 4 * H], BF16)
            nc.sync.dma_start(wh_sb[:], wht)
            if has_bias:
                b_sb = const.tile([H, 4], F32)
                nc.sync.dma_start(b_sb[:], bias)
            nc.sync.dma_start(ca[:], ct[:, 0:SW])
            nc.sync.dma_start(cla[:], clt[:, 0:SW])
            nc.sync.dma_start(xb[:], xt[:, SW:BC])
            nc.sync.dma_start(hb[:], ht[:, SW:BC])
            nc.sync.dma_start(cb[:], ct[:, SW:BC])
            nc.sync.dma_start(clb[:], clt[:, SW:BC])

            # ACT table preload (sigmoid set includes tanh) overlaps DMA fill
            dummy = const.tile([H, 8], F32)
            nc.vector.memset(dummy[:], 0.0)
            dummy2 = const.tile([H, 8], F32)
            nc.scalar.activation(dummy2[:], dummy[:], AF.Sigmoid)

            def in_slice(t_small, t_big, g, w):
                c0 = g * GW
                if c0 + w <= SW:
                    return t_small[:, c0:c0 + w]
                return t_big[:, c0 - SW:c0 - SW + w]

            # HAM warmup on a junk tile while DMAs stream
            junk = const.tile([H, GW], BF16)
            nc.vector.memset(junk[:], 0.0)
            warm = qp.tile([H, 2048], F32, name="warm", tag="quad")
            for _ in range(8):
                nc.tensor.matmul(warm[:, 0:GW], junk[:, 0:H], junk[:],
                                 start=True, stop=True)

            NP = NG // 2  # pairs
            sig2s = {}

            # pair -> (chunk_start_group, chunk_width, local_offset, is_last)
            pair_chunk = {}
            for cs, cw in OCHUNKS:
                for g in range(cs, cs + cw, 2):
                    pair_chunk[g // 2] = (cs, cw * GW, (g - cs) * GW,
                                          g + 2 == cs + cw)

            def emit_tanh_h(P):
                """tanh + h' + (maybe) hn DMA for pair P (c' already done)."""
                cs, cw, lo, last = pair_chunk[P]
                cnb, hnb = cn_hn[P]
                tcp = tp.tile([H, 1024], BF16, name=f"tc{P}", tag="tc")
                nc.scalar.activation(tcp[:], cnb[:, lo:lo + 2 * GW], AF.Tanh)
                sig2 = sig2s.pop(P)
                o3 = sig2[:].rearrange("p (t x) -> p t x", t=2)[:, :, 1024:1536]
                h3 = hnb[:, lo:lo + 2 * GW].rearrange("p (t x) -> p t x", t=2)
                t3 = tcp[:].rearrange("p (t x) -> p t x", t=2)
                nc.vector.tensor_mul(h3, o3, t3)
                if last:
                    nc.sync.dma_start(hnt[:, cs * GW:cs * GW + cw], hnb[:])

            cn_hn = {}
            cn_buf = hn_buf = None
            for P in range(NP):
                g0 = 2 * P
                cs, cw, lo, last = pair_chunk[P]
                if lo == 0:
                    cn_buf = op.tile([H, cw], BF16, name=f"cn{g0}", tag="cn")
                    hn_buf = op.tile([H, cw], BF16, name=f"hn{g0}", tag="hn")
                cn_hn[P] = (cn_buf, hn_buf)
                sig2 = sp.tile([H, 4096], BF16, name=f"s{P}", tag="sig")
                sig2s[P] = sig2

                for gg in range(2):
                    g = g0 + gg
                    xs = in_slice(xa, xb, g, GW)
                    hs = in_slice(ha, hb, g, GW)
                    quad = qp.tile([H, 2048], F32, name=f"q{g}", tag="quad")
                    for k in range(4):
                        nc.tensor.matmul(quad[:, k * GW:(k + 1) * GW],
                                         wx_sb[:, k * H:(k + 1) * H], xs,
                                         start=True, stop=False)
                        nc.tensor.matmul(quad[:, k * GW:(k + 1) * GW],
                                         wh_sb[:, k * H:(k + 1) * H], hs,
                                         start=False, stop=True)
                    so = sig2[:, gg * 2048:(gg + 1) * 2048]
                    if has_bias:
                        for k in range(4):
                            nc.scalar.activation(
                                so[:, k * GW:(k + 1) * GW],
                                quad[:, k * GW:(k + 1) * GW],
                                AF.Sigmoid, bias=b_sb[:, k:k + 1])
                    else:
                        nc.scalar.activation(so, quad[:], AF.Sigmoid)
                    if gg == 1 and P >= 1:
                        emit_tanh_h(P - 1)

                # DVE chain for pair P (3D APs batch both groups, 2x mode).
                # c is carried as bf16 hi+lo so the dominant c-quantization
                # error cancels: c' = (ig + f*c_hi) + f*c_lo.
                s3 = sig2[:].rearrange("p (t x) -> p t x", t=2)
                i3 = s3[:, :, 0:512]
                f3 = s3[:, :, 512:1024]
                sg3 = s3[:, :, 1536:2048]
                c3 = in_slice(ca, cb, g0, 1024).rearrange(
                    "p (t x) -> p t x", t=2)
                cl3 = in_slice(cla, clb, g0, 1024).rearrange(
                    "p (t x) -> p t x", t=2)
                gt = tp.tile([H, 1024], BF16, name=f"gt{P}", tag="gt")
                gt3 = gt[:].rearrange("p (t x) -> p t x", t=2)
                nc.vector.tensor_scalar(gt3, sg3, 2.0, 1.0,
                                        ALU.mult, ALU.subtract)
                ig = tp.tile([H, 1024], BF16, name=f"ig{P}", tag="ig")
                ig3 = ig[:].rearrange("p (t x) -> p t x", t=2)
                nc.vector.tensor_mul(ig3, i3, gt3)
                fch = tp.tile([H, 1024], BF16, name=f"fch{P}", tag="fch")
                fch3 = fch[:].rearrange("p (t x) -> p t x", t=2)
                nc.vector.tensor_mul(fch3, f3, c3)
                fcl = tp.tile([H, 1024], BF16, name=f"fcl{P}", tag="fcl")
                fcl3 = fcl[:].rearrange("p (t x) -> p t x", t=2)
                nc.vector.tensor_mul(fcl3, f3, cl3)
                v = tp.tile([H, 1024], BF16, name=f"v{P}", tag="v")
                nc.vector.tensor_add(v[:], ig[:], fch[:])
                cns = cn_buf[:, lo:lo + 2 * GW]
                nc.vector.tensor_add(cns, v[:], fcl[:])
                if last:
                    nc.sync.dma_start(cnt[:, cs * GW:cs * GW + cw], cn_buf[:])

            emit_tanh_h(NP - 1)
    nc.compile()
    return nc


def _run(inputs, trace=False, tmpdir=None):
    x = np.asarray(inputs["x"], dtype=np.float32)
    h = np.asarray(inputs["h_t"], dtype=np.float32)
    c = np.asarray(inputs["c_t"], dtype=np.float32)
    # gate order [i, f, o, g]; W_g/b_g scaled by 2 for the tanh-via-sigmoid
    wx = np.concatenate([inputs["W_ii"], inputs["W_if"], inputs["W_io"],
                         2.0 * np.asarray(inputs["W_ig"])], axis=0)
    wh = np.concatenate([inputs["W_hi"], inputs["W_hf"], inputs["W_ho"],
                         2.0 * np.asarray(inputs["W_hg"])], axis=0)
    b = np.concatenate([inputs["b_i"], inputs["b_f"], inputs["b_o"],
                        2.0 * np.asarray(inputs["b_g"])], axis=0)
    wxt = np.ascontiguousarray(wx.T).astype(NPDT)
    wht = np.ascontiguousarray(wh.T).astype(NPDT)
    has_bias = bool(np.any(b))

    key = has_bias
    if key not in _CACHE:
        _CACHE[key] = _build(has_bias)
    nc = _CACHE[key]

    x16 = x.astype(NPDT)
    h16 = h.astype(NPDT)
    c16 = c.astype(NPDT)
    cl16 = (c - c16.astype(np.float32)).astype(NPDT)
    in_maps = []
    for i in range(NCORES):
        s = slice(i * BC, (i + 1) * BC)
        m = {
            "xt": np.ascontiguousarray(x16[s].T),
            "ht": np.ascontiguousarray(h16[s].T),
            "ct": np.ascontiguousarray(c16[s].T),
            "clt": np.ascontiguousarray(cl16[s].T),
            "wxt": wxt,
            "wht": wht,
        }
        if has_bias:
            m["bias"] = np.ascontiguousarray(
                b.reshape(4, H).T.astype(np.float32))
        in_maps.append(m)

    res = run_bass_kernel_spmd(nc, in_maps, core_ids=list(range(NCORES)),
                               trace=trace, tmpdir=tmpdir)
    h_new = np.empty((NCORES * BC, H), dtype=np.float32)
    c_new = np.empty((NCORES * BC, H), dtype=np.float32)
    for i, r in enumerate(res.results):
        s = slice(i * BC, (i + 1) * BC)
        h_new[s] = r["hnt"].T
        c_new[s] = r["cnt"].T
    return h_new, c_new, res


def kernel(**inputs):
    h_new, c_new, _ = _run(inputs, trace=False)
    return h_new, c_new


# revision 19
# speedup vs baseline: 1.4442x; 1.0747x over previous
"""LSTMCell on 8 Trainium2 NeuronCores, data-parallel over the batch.

Full inputs: x/h_t/c_t [65536,128] f32, 8 gate weight matrices [128,128],
4 biases [128]. Returns (h_new, c_new) as [65536,128] f32 each.

v3 design (all-bf16, transposed layout, zero on-device transposes):
  - Host transposes x/h/c per core to xT/hT/cT [128 feat, 8192 batch] bf16
    and pre-concats weights as WxT/WhT [128 in, 512 gate-rows] bf16 in gate
    order [i, f, o, 2*g] (g prescaled by 2 for the tanh-via-sigmoid trick).
  - Per batch group of 512 columns: 8 bf16 matmuls (weights stationary,
    batch streams; PE issue period ~259ns) accumulate gates^T into a 4-bank
    PSUM quad [128, 2048]: banks = i|f|o|2g pre-acts.
  - ONE sigmoid per quad -> bf16 SBUF (bf16 ACT output is ~1.7x faster than
    fp16 out); two groups share a sig2 tile [128, 4096] so DVE ops batch
    per PAIR of groups via 3D APs (keeps the DVE 2x bf16 mode).
  - DVE per pair: gt=2s-1 [TS 4x], ig=i*gt [TT], fc=f*c [TT],
    c'=ig+fc [TT], h'=o*tanh(c') [TT].
  - tanh(c') on ACT, delayed one pair so ACT never stalls on the DVE chain.
  - DMA: inputs split 4+12 groups (fast fill + big descriptors), outputs
    per 8 groups (1MB, 8KB/partition descriptors).
"""
import numpy as np
import ml_dtypes
from contextlib import ExitStack

import concourse.bass as bass
import concourse.tile as tile
from concourse import bacc, mybir
from concourse.bass_utils import run_bass_kernel_spmd

F32 = mybir.dt.float32
F16 = mybir.dt.float16
BF16 = mybir.dt.bfloat16
NPBF = ml_dtypes.bfloat16
AF = mybir.ActivationFunctionType
ALU = mybir.AluOpType

NCORES = 8
BC = 8192            # batch rows per core
GW = 512             # batch columns per group (one PSUM bank)
NG = BC // GW        # 16 groups
H = 128              # hidden size
# input chunks in groups: small (fast fill), medium, large
ICHUNKS = [(0, 2), (2, 6), (8, 8)]
# output chunks (start group, n groups): big, medium, small tail
OCHUNKS = [(0, 8), (8, 6), (14, 2)]

_CACHE = {}


def _build(has_bias: bool):
    nc = bacc.Bacc("TRN2", target_bir_lowering=False, debug=False)
    xt = nc.dram_tensor("xt", [H, BC], F16, kind="ExternalInput").ap()
    ht = nc.dram_tensor("ht", [H, BC], F16, kind="ExternalInput").ap()
    ct = nc.dram_tensor("ct", [H, BC], BF16, kind="ExternalInput").ap()
    clt = nc.dram_tensor("clt", [H, BC], BF16, kind="ExternalInput").ap()
    wxt = nc.dram_tensor("wxt", [H, 4 * H], F16, kind="ExternalInput").ap()
    wht = nc.dram_tensor("wht", [H, 4 * H], F16, kind="ExternalInput").ap()
    if has_bias:
        bias = nc.dram_tensor("bias", [H, 4], F32, kind="ExternalInput").ap()
    hnt = nc.dram_tensor("hnt", [H, BC], BF16, kind="ExternalOutput").ap()
    cnt = nc.dram_tensor("cnt", [H, BC], BF16, kind="ExternalOutput").ap()



    with tile.TileContext(nc) as tc:
        with ExitStack() as ctx:
            const = ctx.enter_context(tc.tile_pool(name="const", bufs=1))
            ina = ctx.enter_context(tc.tile_pool(name="ina", bufs=1))
            qp = ctx.enter_context(tc.tile_pool(name="qp", bufs=2, space="PSUM"))
            sp = ctx.enter_context(tc.tile_pool(name="sp", bufs=3))
            tp = ctx.enter_context(tc.tile_pool(name="tp", bufs=2))
            op = ctx.enter_context(tc.tile_pool(name="op", bufs=2))

            # Input tiles in 3 chunks per tensor: small chunk first for fast
            # pipeline fill, then medium/large for DMA efficiency.  x/h
            # chunks issue before c (c is consumed later, by the DVE chain).
            xts, hts, cts, clts = [], [], [], []
            for ci, (cs, cw) in enumerate(ICHUNKS):
                for lst, nm in ((xts, "x"), (hts, "h"), (cts, "c"),
                                (clts, "cl")):
                    dt = F16 if nm in ("x", "h") else BF16
                    lst.append(ina.tile([H, cw * GW], dt, name=f"{nm}{ci}"))
            for ci, (cs, cw) in enumerate(ICHUNKS):
                c0, c1 = cs * GW, (cs + cw) * GW
                nc.sync.dma_start(xts[ci][:], xt[:, c0:c1])
                nc.sync.dma_start(hts[ci][:], ht[:, c0:c1])
                if ci == 0:
                    wx_sb = const.tile([H, 4 * H], F16)
                    nc.sync.dma_start(wx_sb[:], wxt)
                    wh_sb = const.tile([H, 4 * H], F16)
                    nc.sync.dma_start(wh_sb[:], wht)
                    if has_bias:
                        b_sb = const.tile([H, 4], F32)
                        nc.sync.dma_start(b_sb[:], bias)
                nc.sync.dma_start(cts[ci][:], ct[:, c0:c1])
                nc.sync.dma_start(clts[ci][:], clt[:, c0:c1])

            # ACT table preload (sigmoid set includes tanh) overlaps DMA fill
            dummy = const.tile([H, 8], F32)
            nc.vector.memset(dummy[:], 0.0)
            dummy2 = const.tile([H, 8], F32)
            nc.scalar.activation(dummy2[:], dummy[:], AF.Sigmoid)

            def in_slice(tiles, g, w):
                c0 = g * GW
                for ci, (cs, cw) in enumerate(ICHUNKS):
                    if c0 + w <= (cs + cw) * GW:
                        return tiles[ci][:, c0 - cs * GW:c0 - cs * GW + w]
                raise AssertionError("slice straddles input chunks")

            # HAM warmup on a junk tile while DMAs stream
            junk = const.tile([H, GW], F16)
            nc.vector.memset(junk[:], 0.0)
            warm = qp.tile([H, 2048], F32, name="warm", tag="quad")
            for _ in range(5):
                nc.tensor.matmul(warm[:, 0:GW], junk[:, 0:H], junk[:],
                                 start=True, stop=True)

            NP = NG // 2  # pairs
            sig2s = {}

            # pair -> (chunk_start_group, chunk_width, local_offset, is_last)
            pair_chunk = {}
            for cs, cw in OCHUNKS:
                for g in range(cs, cs + cw, 2):
                    pair_chunk[g // 2] = (cs, cw * GW, (g - cs) * GW,
                                          g + 2 == cs + cw)

            def emit_tanh_h(P):
                """tanh + h' + (maybe) hn DMA for pair P (c' already done)."""
                cs, cw, lo, last = pair_chunk[P]
                cnb, hnb = cn_hn[P]
                tcp = tp.tile([H, 1024], BF16, name=f"tc{P}", tag="tc")
                nc.scalar.activation(tcp[:], cnb[:, lo:lo + 2 * GW], AF.Tanh)
                sig2 = sig2s.pop(P)
                o3 = sig2[:].rearrange("p (t x) -> p t x", t=2)[:, :, 1024:1536]
                h3 = hnb[:, lo:lo + 2 * GW].rearrange("p (t x) -> p t x", t=2)
                t3 = tcp[:].rearrange("p (t x) -> p t x", t=2)
                nc.vector.tensor_mul(h3, o3, t3)
                if last:
                    nc.sync.dma_start(hnt[:, cs * GW:cs * GW + cw], hnb[:])

            cn_hn = {}
            cn_buf = hn_buf = None
            for P in range(NP):
                g0 = 2 * P
                cs, cw, lo, last = pair_chunk[P]
                if lo == 0:
                    cn_buf = op.tile([H, cw], BF16, name=f"cn{g0}", tag="cn")
                    hn_buf = op.tile([H, cw], BF16, name=f"hn{g0}", tag="hn")
                cn_hn[P] = (cn_buf, hn_buf)
                sig2 = sp.tile([H, 4096], BF16, name=f"s{P}", tag="sig")
                sig2s[P] = sig2

                def emit_dve(g_first, ng, tag_sfx):
                    """c'-chain for ng groups starting at g_first (in pair P).
                    c is bf16 hi+lo so the c-quantization error cancels:
                    c' = (ig + f*c_hi) + f*c_lo."""
                    w = ng * GW
                    gg = g_first - g0

                    def sl(bank):
                        s = sig2[:].rearrange("p (t x) -> p t x", t=2)
                        s = s[:, gg:gg + ng, bank * GW:(bank + 1) * GW]
                        return s

                    def r3(ap2d):
                        return ap2d.rearrange("p (t x) -> p t x", t=ng)

                    c3 = r3(in_slice(cts, g_first, w))
                    cl3 = r3(in_slice(clts, g_first, w))
                    gt = tp.tile([H, w], BF16, name=f"gt{tag_sfx}", tag="gt")
                    nc.vector.tensor_scalar(r3(gt[:]), sl(0 + 3), 2.0, 1.0,
                                            ALU.mult, ALU.subtract)
                    ig = tp.tile([H, w], BF16, name=f"ig{tag_sfx}", tag="ig")
                    nc.vector.tensor_mul(r3(ig[:]), sl(0), r3(gt[:]))
                    fch = tp.tile([H, w], BF16, name=f"fch{tag_sfx}",
                                  tag="fch")
                    nc.vector.tensor_mul(r3(fch[:]), sl(1), c3)
                    fcl = tp.tile([H, w], BF16, name=f"fcl{tag_sfx}",
                                  tag="fcl")
                    nc.vector.tensor_mul(r3(fcl[:]), sl(1), cl3)
                    v = tp.tile([H, w], BF16, name=f"v{tag_sfx}", tag="v")
                    nc.vector.tensor_add(v[:], ig[:], fch[:])
                    lg = lo + gg * GW
                    nc.vector.tensor_add(cn_buf[:, lg:lg + w], v[:], fcl[:])
                    if last and gg + ng == 2:
                        nc.sync.dma_start(
                            cnt[:, cs * GW:cs * GW + cw], cn_buf[:])

                lastP = P == NP - 1
                for gg in range(2):
                    g = g0 + gg
                    xs = in_slice(xts, g, GW)
                    hs = in_slice(hts, g, GW)
                    quad = qp.tile([H, 2048], F32, name=f"q{g}", tag="quad")
                    for k in range(4):
                        nc.tensor.matmul(quad[:, k * GW:(k + 1) * GW],
                                         wx_sb[:, k * H:(k + 1) * H], xs,
                                         start=True, stop=False)
                        nc.tensor.matmul(quad[:, k * GW:(k + 1) * GW],
                                         wh_sb[:, k * H:(k + 1) * H], hs,
                                         start=False, stop=True)
                    so = sig2[:, gg * 2048:(gg + 1) * 2048]
                    if has_bias:
                        for k in range(4):
                            nc.scalar.activation(
                                so[:, k * GW:(k + 1) * GW],
                                quad[:, k * GW:(k + 1) * GW],
                                AF.Sigmoid, bias=b_sb[:, k:k + 1])
                    else:
                        nc.scalar.activation(so, quad[:], AF.Sigmoid)
                    if lastP:
                        # per-group chain shortens the kernel tail
                        emit_dve(g, 1, f"p{P}g{gg}")
                    if gg == 1 and P >= 1:
                        emit_tanh_h(P - 1)

                if not lastP:
                    emit_dve(g0, 2, f"p{P}")

            emit_tanh_h(NP - 1)
    nc.compile()
    return nc


def _run(inputs, trace=False, tmpdir=None):
    x = np.asarray(inputs["x"], dtype=np.float32)
    h = np.asarray(inputs["h_t"], dtype=np.float32)
    c = np.asarray(inputs["c_t"], dtype=np.float32)
    # gate order [i, f, o, g]; W_g/b_g scaled by 2 for the tanh-via-sigmoid
    wx = np.concatenate([inputs["W_ii"], inputs["W_if"], inputs["W_io"],
                         2.0 * np.asarray(inputs["W_ig"])], axis=0)
    wh = np.concatenate([inputs["W_hi"], inputs["W_hf"], inputs["W_ho"],
                         2.0 * np.asarray(inputs["W_hg"])], axis=0)
    b = np.concatenate([inputs["b_i"], inputs["b_f"], inputs["b_o"],
                        2.0 * np.asarray(inputs["b_g"])], axis=0)
    wxt = np.ascontiguousarray(wx.T).astype(np.float16)
    wht = np.ascontiguousarray(wh.T).astype(np.float16)
    has_bias = bool(np.any(b))

    key = has_bias
    if key not in _CACHE:
        _CACHE[key] = _build(has_bias)
    nc = _CACHE[key]

    x16 = x.astype(np.float16)
    h16 = h.astype(np.float16)
    c16 = c.astype(NPBF)
    cl16 = (c - c16.astype(np.float32)).astype(NPBF)
    in_maps = []
    for i in range(NCORES):
        s = slice(i * BC, (i + 1) * BC)
        m = {
            "xt": np.ascontiguousarray(x16[s].T),
            "ht": np.ascontiguousarray(h16[s].T),
            "ct": np.ascontiguousarray(c16[s].T),
            "clt": np.ascontiguousarray(cl16[s].T),
            "wxt": wxt,
            "wht": wht,
        }
        if has_bias:
            m["bias"] = np.ascontiguousarray(
                b.reshape(4, H).T.astype(np.float32))
        in_maps.append(m)

    res = run_bass_kernel_spmd(nc, in_maps, core_ids=list(range(NCORES)),
                               trace=trace, tmpdir=tmpdir)
    h_new = np.empty((NCORES * BC, H), dtype=np.float32)
    c_new = np.empty((NCORES * BC, H), dtype=np.float32)
    for i, r in enumerate(res.results):
        s = slice(i * BC, (i + 1) * BC)
        h_new[s] = r["hnt"].T
        c_new[s] = r["cnt"].T
    return h_new, c_new, res


def kernel(**inputs):
    h_new, c_new, _ = _run(inputs, trace=False)
    return h_new, c_new


# revision 20
# speedup vs baseline: 1.4793x; 1.0244x over previous
"""LSTMCell on 8 Trainium2 NeuronCores, data-parallel over the batch.

Full inputs: x/h_t/c_t [65536,128] f32, 8 gate weight matrices [128,128],
4 biases [128]. Returns (h_new, c_new) as [65536,128] f32 each.

v3 design (all-bf16, transposed layout, zero on-device transposes):
  - Host transposes x/h/c per core to xT/hT/cT [128 feat, 8192 batch] bf16
    and pre-concats weights as WxT/WhT [128 in, 512 gate-rows] bf16 in gate
    order [i, f, o, 2*g] (g prescaled by 2 for the tanh-via-sigmoid trick).
  - Per batch group of 512 columns: 8 bf16 matmuls (weights stationary,
    batch streams; PE issue period ~259ns) accumulate gates^T into a 4-bank
    PSUM quad [128, 2048]: banks = i|f|o|2g pre-acts.
  - ONE sigmoid per quad -> bf16 SBUF (bf16 ACT output is ~1.7x faster than
    fp16 out); two groups share a sig2 tile [128, 4096] so DVE ops batch
    per PAIR of groups via 3D APs (keeps the DVE 2x bf16 mode).
  - DVE per pair: gt=2s-1 [TS 4x], ig=i*gt [TT], fc=f*c [TT],
    c'=ig+fc [TT], h'=o*tanh(c') [TT].
  - tanh(c') on ACT, delayed one pair so ACT never stalls on the DVE chain.
  - DMA: inputs split 4+12 groups (fast fill + big descriptors), outputs
    per 8 groups (1MB, 8KB/partition descriptors).
"""
import numpy as np
import ml_dtypes
from contextlib import ExitStack

import concourse.bass as bass
import concourse.tile as tile
from concourse import bacc, mybir
from concourse.bass_utils import run_bass_kernel_spmd

F32 = mybir.dt.float32
F16 = mybir.dt.float16
BF16 = mybir.dt.bfloat16
NPBF = ml_dtypes.bfloat16
AF = mybir.ActivationFunctionType
ALU = mybir.AluOpType

NCORES = 8
BC = 8192            # batch rows per core
GW = 512             # batch columns per group (one PSUM bank)
NG = BC // GW        # 16 groups
H = 128              # hidden size
# input chunks in groups: small (fast fill), medium, large
ICHUNKS = [(0, 2), (2, 6), (8, 8)]
# output chunks (start group, n groups): big, medium, small tail
OCHUNKS = [(0, 8), (8, 6), (14, 2)]

_CACHE = {}


def _build(has_bias: bool):
    nc = bacc.Bacc("TRN2", target_bir_lowering=False, debug=False)
    xt = nc.dram_tensor("xt", [H, BC], F16, kind="ExternalInput").ap()
    ht = nc.dram_tensor("ht", [H, BC], F16, kind="ExternalInput").ap()
    ct = nc.dram_tensor("ct", [H, BC], F16, kind="ExternalInput").ap()
    wxt = nc.dram_tensor("wxt", [H, 4 * H], F16, kind="ExternalInput").ap()
    wht = nc.dram_tensor("wht", [H, 4 * H], F16, kind="ExternalInput").ap()
    if has_bias:
        bias = nc.dram_tensor("bias", [H, 4], F32, kind="ExternalInput").ap()
    hnt = nc.dram_tensor("hnt", [H, BC], F16, kind="ExternalOutput").ap()
    cnt = nc.dram_tensor("cnt", [H, BC], F16, kind="ExternalOutput").ap()



    with tile.TileContext(nc) as tc:
        with ExitStack() as ctx:
            const = ctx.enter_context(tc.tile_pool(name="const", bufs=1))
            ina = ctx.enter_context(tc.tile_pool(name="ina", bufs=1))
            qp = ctx.enter_context(tc.tile_pool(name="qp", bufs=2, space="PSUM"))
            sp = ctx.enter_context(tc.tile_pool(name="sp", bufs=3))
            tp = ctx.enter_context(tc.tile_pool(name="tp", bufs=2))
            op = ctx.enter_context(tc.tile_pool(name="op", bufs=2))

            # Input tiles in 3 chunks per tensor: small chunk first for fast
            # pipeline fill, then medium/large for DMA efficiency.  x/h
            # chunks issue before c (c is consumed later, by the DVE chain).
            xts, hts, cts = [], [], []
            for ci, (cs, cw) in enumerate(ICHUNKS):
                for lst, nm in ((xts, "x"), (hts, "h"), (cts, "c")):
                    lst.append(ina.tile([H, cw * GW], F16,
                                        name=f"{nm}{ci}"))
            for ci, (cs, cw) in enumerate(ICHUNKS):
                c0, c1 = cs * GW, (cs + cw) * GW
                nc.sync.dma_start(xts[ci][:], xt[:, c0:c1])
                nc.sync.dma_start(hts[ci][:], ht[:, c0:c1])
                if ci == 0:
                    wx_sb = const.tile([H, 4 * H], F16)
                    nc.sync.dma_start(wx_sb[:], wxt)
                    wh_sb = const.tile([H, 4 * H], F16)
                    nc.sync.dma_start(wh_sb[:], wht)
                    if has_bias:
                        b_sb = const.tile([H, 4], F32)
                        nc.sync.dma_start(b_sb[:], bias)
                nc.sync.dma_start(cts[ci][:], ct[:, c0:c1])

            # ACT table preload (sigmoid set includes tanh) overlaps DMA fill
            dummy = const.tile([H, 8], F32)
            nc.vector.memset(dummy[:], 0.0)
            dummy2 = const.tile([H, 8], F32)
            nc.scalar.activation(dummy2[:], dummy[:], AF.Sigmoid)

            def in_slice(tiles, g, w):
                c0 = g * GW
                for ci, (cs, cw) in enumerate(ICHUNKS):
                    if c0 + w <= (cs + cw) * GW:
                        return tiles[ci][:, c0 - cs * GW:c0 - cs * GW + w]
                raise AssertionError("slice straddles input chunks")

            # HAM warmup on a junk tile while DMAs stream
            junk = const.tile([H, GW], F16)
            nc.vector.memset(junk[:], 0.0)
            warm = qp.tile([H, 2048], F32, name="warm", tag="quad")
            for _ in range(5):
                nc.tensor.matmul(warm[:, 0:GW], junk[:, 0:H], junk[:],
                                 start=True, stop=True)

            NP = NG // 2  # pairs
            sig2s = {}

            # pair -> (chunk_start_group, chunk_width, local_offset, is_last)
            pair_chunk = {}
            for cs, cw in OCHUNKS:
                for g in range(cs, cs + cw, 2):
                    pair_chunk[g // 2] = (cs, cw * GW, (g - cs) * GW,
                                          g + 2 == cs + cw)

            def emit_tanh_h(P):
                """tanh + h' + (maybe) hn DMA for pair P (c' already done)."""
                cs, cw, lo, last = pair_chunk[P]
                cnb, hnb = cn_hn[P]
                tcp = tp.tile([H, 1024], BF16, name=f"tc{P}", tag="tc")
                nc.scalar.activation(tcp[:], cnb[:, lo:lo + 2 * GW], AF.Tanh)
                sig2 = sig2s.pop(P)
                o3 = sig2[:].rearrange("p (t x) -> p t x", t=2)[:, :, 1024:1536]
                h3 = hnb[:, lo:lo + 2 * GW].rearrange("p (t x) -> p t x", t=2)
                t3 = tcp[:].rearrange("p (t x) -> p t x", t=2)
                nc.vector.tensor_mul(h3, o3, t3)
                if last:
                    nc.sync.dma_start(hnt[:, cs * GW:cs * GW + cw], hnb[:])

            cn_hn = {}
            cn_buf = hn_buf = None
            for P in range(NP):
                g0 = 2 * P
                cs, cw, lo, last = pair_chunk[P]
                if lo == 0:
                    cn_buf = op.tile([H, cw], F16, name=f"cn{g0}", tag="cn")
                    hn_buf = op.tile([H, cw], F16, name=f"hn{g0}", tag="hn")
                cn_hn[P] = (cn_buf, hn_buf)
                sig2 = sp.tile([H, 4096], BF16, name=f"s{P}", tag="sig")
                sig2s[P] = sig2

                def emit_dve(g_first, ng, tag_sfx):
                    """c'-chain for ng groups starting at g_first (pair P).
                    ig/fc/c' are fp16: bf16 rounding of the large ig/fc
                    terms would dominate the error after cancellation."""
                    w = ng * GW
                    gg = g_first - g0

                    def sl(bank):
                        s = sig2[:].rearrange("p (t x) -> p t x", t=2)
                        s = s[:, gg:gg + ng, bank * GW:(bank + 1) * GW]
                        return s

                    def r3(ap2d):
                        return ap2d.rearrange("p (t x) -> p t x", t=ng)

                    c3 = r3(in_slice(cts, g_first, w))
                    gt = tp.tile([H, w], BF16, name=f"gt{tag_sfx}", tag="gt")
                    nc.vector.tensor_scalar(r3(gt[:]), sl(0 + 3), 2.0, 1.0,
                                            ALU.mult, ALU.subtract)
                    ig = tp.tile([H, w], F16, name=f"ig{tag_sfx}", tag="ig")
                    nc.vector.tensor_mul(r3(ig[:]), sl(0), r3(gt[:]))
                    fc = tp.tile([H, w], F16, name=f"fc{tag_sfx}", tag="fc")
                    nc.vector.tensor_mul(r3(fc[:]), sl(1), c3)
                    lg = lo + gg * GW
                    nc.vector.tensor_add(cn_buf[:, lg:lg + w], ig[:], fc[:])
                    if last and gg + ng == 2:
                        nc.sync.dma_start(
                            cnt[:, cs * GW:cs * GW + cw], cn_buf[:])

                lastP = P == NP - 1
                for gg in range(2):
                    g = g0 + gg
                    xs = in_slice(xts, g, GW)
                    hs = in_slice(hts, g, GW)
                    quad = qp.tile([H, 2048], F32, name=f"q{g}", tag="quad")
                    for k in range(4):
                        nc.tensor.matmul(quad[:, k * GW:(k + 1) * GW],
                                         wx_sb[:, k * H:(k + 1) * H], xs,
                                         start=True, stop=False)
                        nc.tensor.matmul(quad[:, k * GW:(k + 1) * GW],
                                         wh_sb[:, k * H:(k + 1) * H], hs,
                                         start=False, stop=True)
                    so = sig2[:, gg * 2048:(gg + 1) * 2048]
                    if has_bias:
                        for k in range(4):
                            nc.scalar.activation(
                                so[:, k * GW:(k + 1) * GW],
                                quad[:, k * GW:(k + 1) * GW],
                                AF.Sigmoid, bias=b_sb[:, k:k + 1])
                    else:
                        nc.scalar.activation(so, quad[:], AF.Sigmoid)
                    if lastP:
                        # per-group chain shortens the kernel tail
                        emit_dve(g, 1, f"p{P}g{gg}")
                    if gg == 1 and P >= 1:
                        emit_tanh_h(P - 1)

                if not lastP:
                    emit_dve(g0, 2, f"p{P}")

            emit_tanh_h(NP - 1)
    nc.compile()
    return nc


def _run(inputs, trace=False, tmpdir=None):
    x = np.asarray(inputs["x"], dtype=np.float32)
    h = np.asarray(inputs["h_t"], dtype=np.float32)
    c = np.asarray(inputs["c_t"], dtype=np.float32)
    # gate order [i, f, o, g]; W_g/b_g scaled by 2 for the tanh-via-sigmoid
    wx = np.concatenate([inputs["W_ii"], inputs["W_if"], inputs["W_io"],
                         2.0 * np.asarray(inputs["W_ig"])], axis=0)
    wh = np.concatenate([inputs["W_hi"], inputs["W_hf"], inputs["W_ho"],
                         2.0 * np.asarray(inputs["W_hg"])], axis=0)
    b = np.concatenate([inputs["b_i"], inputs["b_f"], inputs["b_o"],
                        2.0 * np.asarray(inputs["b_g"])], axis=0)
    wxt = np.ascontiguousarray(wx.T).astype(np.float16)
    wht = np.ascontiguousarray(wh.T).astype(np.float16)
    has_bias = bool(np.any(b))

    key = has_bias
    if key not in _CACHE:
        _CACHE[key] = _build(has_bias)
    nc = _CACHE[key]

    x16 = x.astype(np.float16)
    h16 = h.astype(np.float16)
    c16 = c.astype(np.float16)
    in_maps = []
    for i in range(NCORES):
        s = slice(i * BC, (i + 1) * BC)
        m = {
            "xt": np.ascontiguousarray(x16[s].T),
            "ht": np.ascontiguousarray(h16[s].T),
            "ct": np.ascontiguousarray(c16[s].T),
            "wxt": wxt,
            "wht": wht,
        }
        if has_bias:
            m["bias"] = np.ascontiguousarray(
                b.reshape(4, H).T.astype(np.float32))
        in_maps.append(m)

    res = run_bass_kernel_spmd(nc, in_maps, core_ids=list(range(NCORES)),
                               trace=trace, tmpdir=tmpdir)
    h_new = np.empty((NCORES * BC, H), dtype=np.float32)
    c_new = np.empty((NCORES * BC, H), dtype=np.float32)
    for i, r in enumerate(res.results):
        s = slice(i * BC, (i + 1) * BC)
        h_new[s] = r["hnt"].T
        c_new[s] = r["cnt"].T
    return h_new, c_new, res


def kernel(**inputs):
    h_new, c_new, _ = _run(inputs, trace=False)
    return h_new, c_new


# revision 22
# speedup vs baseline: 1.4969x; 1.0118x over previous
"""LSTMCell on 8 Trainium2 NeuronCores, data-parallel over the batch.

Full inputs: x/h_t/c_t [65536,128] f32, 8 gate weight matrices [128,128],
4 biases [128]. Returns (h_new, c_new) as [65536,128] f32 each.

v3 design (all-bf16, transposed layout, zero on-device transposes):
  - Host transposes x/h/c per core to xT/hT/cT [128 feat, 8192 batch] bf16
    and pre-concats weights as WxT/WhT [128 in, 512 gate-rows] bf16 in gate
    order [i, f, o, 2*g] (g prescaled by 2 for the tanh-via-sigmoid trick).
  - Per batch group of 512 columns: 8 bf16 matmuls (weights stationary,
    batch streams; PE issue period ~259ns) accumulate gates^T into a 4-bank
    PSUM quad [128, 2048]: banks = i|f|o|2g pre-acts.
  - ONE sigmoid per quad -> bf16 SBUF (bf16 ACT output is ~1.7x faster than
    fp16 out); two groups share a sig2 tile [128, 4096] so DVE ops batch
    per PAIR of groups via 3D APs (keeps the DVE 2x bf16 mode).
  - DVE per pair: gt=2s-1 [TS 4x], ig=i*gt [TT], fc=f*c [TT],
    c'=ig+fc [TT], h'=o*tanh(c') [TT].
  - tanh(c') on ACT, delayed one pair so ACT never stalls on the DVE chain.
  - DMA: inputs split 4+12 groups (fast fill + big descriptors), outputs
    per 8 groups (1MB, 8KB/partition descriptors).
"""
import numpy as np
import ml_dtypes
from contextlib import ExitStack

import concourse.bass as bass
import concourse.tile as tile
from concourse import bacc, mybir
from concourse.bass_utils import run_bass_kernel_spmd

F32 = mybir.dt.float32
F16 = mybir.dt.float16
BF16 = mybir.dt.bfloat16
NPBF = ml_dtypes.bfloat16
AF = mybir.ActivationFunctionType
ALU = mybir.AluOpType

NCORES = 8
BC = 8192            # batch rows per core
GW = 512             # batch columns per group (one PSUM bank)
NG = BC // GW        # 16 groups
H = 128              # hidden size
# input chunks in groups: small (fast fill), then growing
ICHUNKS = [(0, 1), (1, 3), (4, 6), (10, 6)]
# output chunks (start group, n groups): big, medium, small tail
OCHUNKS = [(0, 8), (8, 6), (14, 2)]

_CACHE = {}


def _build(has_bias: bool):
    nc = bacc.Bacc("TRN2", target_bir_lowering=False, debug=False)
    xt = nc.dram_tensor("xt", [H, BC], F16, kind="ExternalInput").ap()
    ht = nc.dram_tensor("ht", [H, BC], F16, kind="ExternalInput").ap()
    ct = nc.dram_tensor("ct", [H, BC], F16, kind="ExternalInput").ap()
    wxt = nc.dram_tensor("wxt", [H, 4 * H], F16, kind="ExternalInput").ap()
    wht = nc.dram_tensor("wht", [H, 4 * H], F16, kind="ExternalInput").ap()
    if has_bias:
        bias = nc.dram_tensor("bias", [H, 4], F32, kind="ExternalInput").ap()
    hnt = nc.dram_tensor("hnt", [H, BC], F16, kind="ExternalOutput").ap()
    cnt = nc.dram_tensor("cnt", [H, BC], F16, kind="ExternalOutput").ap()



    with tile.TileContext(nc) as tc:
        with ExitStack() as ctx:
            const = ctx.enter_context(tc.tile_pool(name="const", bufs=1))
            ina = ctx.enter_context(tc.tile_pool(name="ina", bufs=1))
            qp = ctx.enter_context(tc.tile_pool(name="qp", bufs=2, space="PSUM"))
            sp = ctx.enter_context(tc.tile_pool(name="sp", bufs=3))
            tp = ctx.enter_context(tc.tile_pool(name="tp", bufs=2))
            op = ctx.enter_context(tc.tile_pool(name="op", bufs=2))

            # Input tiles in 3 chunks per tensor: small chunk first for fast
            # pipeline fill, then medium/large for DMA efficiency.  x/h
            # chunks issue before c (c is consumed later, by the DVE chain).
            xts, hts, cts = [], [], []
            for ci, (cs, cw) in enumerate(ICHUNKS):
                for lst, nm in ((xts, "x"), (hts, "h"), (cts, "c")):
                    lst.append(ina.tile([H, cw * GW], F16,
                                        name=f"{nm}{ci}"))
            def cbounds(ci):
                cs, cw = ICHUNKS[ci]
                return cs * GW, (cs + cw) * GW
            for ci in range(len(ICHUNKS)):
                c0, c1 = cbounds(ci)
                nc.sync.dma_start(xts[ci][:], xt[:, c0:c1])
                nc.sync.dma_start(hts[ci][:], ht[:, c0:c1])
                if ci == 0:
                    wx_sb = const.tile([H, 4 * H], F16)
                    nc.sync.dma_start(wx_sb[:], wxt)
                    wh_sb = const.tile([H, 4 * H], F16)
                    nc.sync.dma_start(wh_sb[:], wht)
                    if has_bias:
                        b_sb = const.tile([H, 4], F32)
                        nc.sync.dma_start(b_sb[:], bias)
                else:
                    c0p, c1p = cbounds(ci - 1)
                    nc.sync.dma_start(cts[ci - 1][:], ct[:, c0p:c1p])
            c0p, c1p = cbounds(len(ICHUNKS) - 1)
            nc.sync.dma_start(cts[-1][:], ct[:, c0p:c1p])

            # ACT table preload (sigmoid set includes tanh) overlaps DMA fill
            dummy = const.tile([H, 8], F32)
            nc.vector.memset(dummy[:], 0.0)
            dummy2 = const.tile([H, 8], F32)
            nc.scalar.activation(dummy2[:], dummy[:], AF.Sigmoid)

            def in_slice(tiles, g, w):
                c0 = g * GW
                for ci, (cs, cw) in enumerate(ICHUNKS):
                    if c0 + w <= (cs + cw) * GW:
                        return tiles[ci][:, c0 - cs * GW:c0 - cs * GW + w]
                raise AssertionError("slice straddles input chunks")

            # HAM warmup on a junk tile while DMAs stream
            junk = const.tile([H, GW], F16)
            nc.vector.memset(junk[:], 0.0)
            warm = qp.tile([H, 2048], F32, name="warm", tag="quad")
            for _ in range(5):
                nc.tensor.matmul(warm[:, 0:GW], junk[:, 0:H], junk[:],
                                 start=True, stop=True)

            NP = NG // 2  # pairs
            sig2s = {}

            # pair -> (chunk_start_group, chunk_width, local_offset, is_last)
            pair_chunk = {}
            for cs, cw in OCHUNKS:
                for g in range(cs, cs + cw, 2):
                    pair_chunk[g // 2] = (cs, cw * GW, (g - cs) * GW,
                                          g + 2 == cs + cw)

            def emit_tanh_h(P):
                """tanh + h' + (maybe) hn DMA for pair P (c' already done)."""
                cs, cw, lo, last = pair_chunk[P]
                cnb, hnb = cn_hn[P]
                tcp = tp.tile([H, 1024], BF16, name=f"tc{P}", tag="tc")
                nc.scalar.activation(tcp[:], cnb[:, lo:lo + 2 * GW], AF.Tanh)
                sig2 = sig2s.pop(P)
                o3 = sig2[:].rearrange("p (t x) -> p t x", t=2)[:, :, 1024:1536]
                h3 = hnb[:, lo:lo + 2 * GW].rearrange("p (t x) -> p t x", t=2)
                t3 = tcp[:].rearrange("p (t x) -> p t x", t=2)
                nc.vector.tensor_mul(h3, o3, t3)
                if last:
                    nc.sync.dma_start(hnt[:, cs * GW:cs * GW + cw], hnb[:])

            cn_hn = {}
            cn_buf = hn_buf = None
            for P in range(NP):
                g0 = 2 * P
                cs, cw, lo, last = pair_chunk[P]
                if lo == 0:
                    cn_buf = op.tile([H, cw], F16, name=f"cn{g0}", tag="cn")
                    hn_buf = op.tile([H, cw], F16, name=f"hn{g0}", tag="hn")
                cn_hn[P] = (cn_buf, hn_buf)
                sig2 = sp.tile([H, 4096], BF16, name=f"s{P}", tag="sig")
                sig2s[P] = sig2

                def emit_dve(g_first, ng, tag_sfx):
                    """c'-chain for ng groups starting at g_first (pair P).
                    ig/fc/c' are fp16: bf16 rounding of the large ig/fc
                    terms would dominate the error after cancellation."""
                    w = ng * GW
                    gg = g_first - g0

                    def sl(bank):
                        s = sig2[:].rearrange("p (t x) -> p t x", t=2)
                        s = s[:, gg:gg + ng, bank * GW:(bank + 1) * GW]
                        return s

                    def r3(ap2d):
                        return ap2d.rearrange("p (t x) -> p t x", t=ng)

                    c3 = r3(in_slice(cts, g_first, w))
                    gt = tp.tile([H, w], BF16, name=f"gt{tag_sfx}", tag="gt")
                    nc.vector.tensor_scalar(r3(gt[:]), sl(0 + 3), 2.0, 1.0,
                                            ALU.mult, ALU.subtract)
                    ig = tp.tile([H, w], F16, name=f"ig{tag_sfx}", tag="ig")
                    nc.vector.tensor_mul(r3(ig[:]), sl(0), r3(gt[:]))
                    fc = tp.tile([H, w], F16, name=f"fc{tag_sfx}", tag="fc")
                    nc.vector.tensor_mul(r3(fc[:]), sl(1), c3)
                    lg = lo + gg * GW
                    nc.vector.tensor_add(cn_buf[:, lg:lg + w], ig[:], fc[:])
                    if last and gg + ng == 2:
                        nc.sync.dma_start(
                            cnt[:, cs * GW:cs * GW + cw], cn_buf[:])

                lastP = P == NP - 1
                for gg in range(2):
                    g = g0 + gg
                    xs = in_slice(xts, g, GW)
                    hs = in_slice(hts, g, GW)
                    quad = qp.tile([H, 2048], F32, name=f"q{g}", tag="quad")
                    for k in range(4):
                        nc.tensor.matmul(quad[:, k * GW:(k + 1) * GW],
                                         wx_sb[:, k * H:(k + 1) * H], xs,
                                         start=True, stop=False)
                        nc.tensor.matmul(quad[:, k * GW:(k + 1) * GW],
                                         wh_sb[:, k * H:(k + 1) * H], hs,
                                         start=False, stop=True)
                    so = sig2[:, gg * 2048:(gg + 1) * 2048]
                    if has_bias:
                        for k in range(4):
                            nc.scalar.activation(
                                so[:, k * GW:(k + 1) * GW],
                                quad[:, k * GW:(k + 1) * GW],
                                AF.Sigmoid, bias=b_sb[:, k:k + 1])
                    else:
                        nc.scalar.activation(so, quad[:], AF.Sigmoid)
                    if lastP or P == 0:
                        # per-group chain: shortens tail (last pair) and
                        # avoids straddling input chunks (first pair)
                        emit_dve(g, 1, f"p{P}g{gg}")
                    if gg == 1 and P >= 1:
                        emit_tanh_h(P - 1)

                if not (lastP or P == 0):
                    emit_dve(g0, 2, f"p{P}")

            # last pair: per-group tanh/h'/hn to shorten the kernel tail
            P = NP - 1
            cs, cw, lo, _ = pair_chunk[P]
            cnb, hnb = cn_hn[P]
            sig2 = sig2s.pop(P)
            for gg in range(2):
                lg = lo + gg * GW
                tcg = tp.tile([H, GW], BF16, name=f"tcz{gg}", tag="tc")
                nc.scalar.activation(tcg[:], cnb[:, lg:lg + GW], AF.Tanh)
                o2 = sig2[:, gg * 2048 + 1024:gg * 2048 + 1536]
                nc.vector.tensor_mul(hnb[:, lg:lg + GW], o2, tcg[:])
                gcol = (cs + gg * (cw // GW - 1)) * GW
                nc.sync.dma_start(hnt[:, gcol:gcol + GW],
                                  hnb[:, lg:lg + GW])
    nc.compile()
    return nc


def _run(inputs, trace=False, tmpdir=None):
    x = np.asarray(inputs["x"], dtype=np.float32)
    h = np.asarray(inputs["h_t"], dtype=np.float32)
    c = np.asarray(inputs["c_t"], dtype=np.float32)
    # gate order [i, f, o, g]; W_g/b_g scaled by 2 for the tanh-via-sigmoid
    wx = np.concatenate([inputs["W_ii"], inputs["W_if"], inputs["W_io"],
                         2.0 * np.asarray(inputs["W_ig"])], axis=0)
    wh = np.concatenate([inputs["W_hi"], inputs["W_hf"], inputs["W_ho"],
                         2.0 * np.asarray(inputs["W_hg"])], axis=0)
    b = np.concatenate([inputs["b_i"], inputs["b_f"], inputs["b_o"],
                        2.0 * np.asarray(inputs["b_g"])], axis=0)
    wxt = np.ascontiguousarray(wx.T).astype(np.float16)
    wht = np.ascontiguousarray(wh.T).astype(np.float16)
    has_bias = bool(np.any(b))

    key = has_bias
    if key not in _CACHE:
        _CACHE[key] = _build(has_bias)
    nc = _CACHE[key]

    x16 = x.astype(np.float16)
    h16 = h.astype(np.float16)
    c16 = c.astype(np.float16)
    in_maps = []
    for i in range(NCORES):
        s = slice(i * BC, (i + 1) * BC)
        m = {
            "xt": np.ascontiguousarray(x16[s].T),
            "ht": np.ascontiguousarray(h16[s].T),
            "ct": np.ascontiguousarray(c16[s].T),
            "wxt": wxt,
            "wht": wht,
        }
        if has_bias:
            m["bias"] = np.ascontiguousarray(
                b.reshape(4, H).T.astype(np.float32))
        in_maps.append(m)

    res = run_bass_kernel_spmd(nc, in_maps, core_ids=list(range(NCORES)),
                               trace=trace, tmpdir=tmpdir)
    h_new = np.empty((NCORES * BC, H), dtype=np.float32)
    c_new = np.empty((NCORES * BC, H), dtype=np.float32)
    for i, r in enumerate(res.results):
        s = slice(i * BC, (i + 1) * BC)
        h_new[s] = r["hnt"].T
        c_new[s] = r["cnt"].T
    return h_new, c_new, res


def kernel(**inputs):
    h_new, c_new, _ = _run(inputs, trace=False)
    return h_new, c_new


# revision 23
# speedup vs baseline: 1.5360x; 1.0261x over previous
"""LSTMCell on 8 Trainium2 NeuronCores, data-parallel over the batch.

Full inputs: x/h_t/c_t [65536,128] f32, 8 gate weight matrices [128,128],
4 biases [128]. Returns (h_new, c_new) as [65536,128] f32 each.

v3 design (all-bf16, transposed layout, zero on-device transposes):
  - Host transposes x/h/c per core to xT/hT/cT [128 feat, 8192 batch] bf16
    and pre-concats weights as WxT/WhT [128 in, 512 gate-rows] bf16 in gate
    order [i, f, o, 2*g] (g prescaled by 2 for the tanh-via-sigmoid trick).
  - Per batch group of 512 columns: 8 bf16 matmuls (weights stationary,
    batch streams; PE issue period ~259ns) accumulate gates^T into a 4-bank
    PSUM quad [128, 2048]: banks = i|f|o|2g pre-acts.
  - ONE sigmoid per quad -> bf16 SBUF (bf16 ACT output is ~1.7x faster than
    fp16 out); two groups share a sig2 tile [128, 4096] so DVE ops batch
    per PAIR of groups via 3D APs (keeps the DVE 2x bf16 mode).
  - DVE per pair: gt=2s-1 [TS 4x], ig=i*gt [TT], fc=f*c [TT],
    c'=ig+fc [TT], h'=o*tanh(c') [TT].
  - tanh(c') on ACT, delayed one pair so ACT never stalls on the DVE chain.
  - DMA: inputs split 4+12 groups (fast fill + big descriptors), outputs
    per 8 groups (1MB, 8KB/partition descriptors).
"""
import numpy as np
import ml_dtypes
from contextlib import ExitStack

import concourse.bass as bass
import concourse.tile as tile
from concourse import bacc, mybir
from concourse.bass_utils import run_bass_kernel_spmd

F32 = mybir.dt.float32
F16 = mybir.dt.float16
BF16 = mybir.dt.bfloat16
NPBF = ml_dtypes.bfloat16
AF = mybir.ActivationFunctionType
ALU = mybir.AluOpType

NCORES = 8
BC = 8192            # batch rows per core
GW = 512             # batch columns per group (one PSUM bank)
NG = BC // GW        # 16 groups
H = 128              # hidden size
# input chunks in groups: small (fast fill), then growing
ICHUNKS = [(0, 1), (1, 1), (2, 2), (4, 4), (8, 8)]
# output chunks (start group, n groups): big, medium, small tail
OCHUNKS = [(0, 8), (8, 6), (14, 2)]

_CACHE = {}


def _build(has_bias: bool):
    nc = bacc.Bacc("TRN2", target_bir_lowering=False, debug=False)
    xt = nc.dram_tensor("xt", [H, BC], F16, kind="ExternalInput").ap()
    ht = nc.dram_tensor("ht", [H, BC], F16, kind="ExternalInput").ap()
    ct = nc.dram_tensor("ct", [H, BC], F16, kind="ExternalInput").ap()
    wxt = nc.dram_tensor("wxt", [H, 4 * H], F16, kind="ExternalInput").ap()
    wht = nc.dram_tensor("wht", [H, 4 * H], F16, kind="ExternalInput").ap()
    if has_bias:
        bias = nc.dram_tensor("bias", [H, 4], F32, kind="ExternalInput").ap()
    hnt = nc.dram_tensor("hnt", [H, BC], F16, kind="ExternalOutput").ap()
    cnt = nc.dram_tensor("cnt", [H, BC], F16, kind="ExternalOutput").ap()



    with tile.TileContext(nc) as tc:
        with ExitStack() as ctx:
            const = ctx.enter_context(tc.tile_pool(name="const", bufs=1))
            ina = ctx.enter_context(tc.tile_pool(name="ina", bufs=1))
            qp = ctx.enter_context(tc.tile_pool(name="qp", bufs=2, space="PSUM"))
            sp = ctx.enter_context(tc.tile_pool(name="sp", bufs=3))
            tp = ctx.enter_context(tc.tile_pool(name="tp", bufs=2))
            op = ctx.enter_context(tc.tile_pool(name="op", bufs=2))

            # Input tiles in 3 chunks per tensor: small chunk first for fast
            # pipeline fill, then medium/large for DMA efficiency.  x/h
            # chunks issue before c (c is consumed later, by the DVE chain).
            xts, hts, cts = [], [], []
            for ci, (cs, cw) in enumerate(ICHUNKS):
                for lst, nm in ((xts, "x"), (hts, "h"), (cts, "c")):
                    lst.append(ina.tile([H, cw * GW], F16,
                                        name=f"{nm}{ci}"))
            def cbounds(ci):
                cs, cw = ICHUNKS[ci]
                return cs * GW, (cs + cw) * GW
            for ci in range(len(ICHUNKS)):
                c0, c1 = cbounds(ci)
                nc.sync.dma_start(xts[ci][:], xt[:, c0:c1])
                nc.sync.dma_start(hts[ci][:], ht[:, c0:c1])
                if ci == 0:
                    wx_sb = const.tile([H, 4 * H], F16)
                    nc.sync.dma_start(wx_sb[:], wxt)
                    wh_sb = const.tile([H, 4 * H], F16)
                    nc.sync.dma_start(wh_sb[:], wht)
                    if has_bias:
                        b_sb = const.tile([H, 4], F32)
                        nc.sync.dma_start(b_sb[:], bias)
                else:
                    c0p, c1p = cbounds(ci - 1)
                    nc.sync.dma_start(cts[ci - 1][:], ct[:, c0p:c1p])
            c0p, c1p = cbounds(len(ICHUNKS) - 1)
            nc.sync.dma_start(cts[-1][:], ct[:, c0p:c1p])

            # ACT table preload (sigmoid set includes tanh) overlaps DMA fill
            dummy = const.tile([H, 8], F32)
            nc.vector.memset(dummy[:], 0.0)
            dummy2 = const.tile([H, 8], F32)
            nc.scalar.activation(dummy2[:], dummy[:], AF.Sigmoid)

            def in_slice(tiles, g, w):
                c0 = g * GW
                for ci, (cs, cw) in enumerate(ICHUNKS):
                    if c0 + w <= (cs + cw) * GW:
                        return tiles[ci][:, c0 - cs * GW:c0 - cs * GW + w]
                raise AssertionError("slice straddles input chunks")

            # HAM warmup on a junk tile while DMAs stream
            junk = const.tile([H, GW], F16)
            nc.vector.memset(junk[:], 0.0)
            warm = qp.tile([H, 2048], F32, name="warm", tag="quad")
            for _ in range(5):
                nc.tensor.matmul(warm[:, 0:GW], junk[:, 0:H], junk[:],
                                 start=True, stop=True)

            NP = NG // 2  # pairs
            sig2s = {}

            # pair -> (chunk_start_group, chunk_width, local_offset, is_last)
            pair_chunk = {}
            for cs, cw in OCHUNKS:
                for g in range(cs, cs + cw, 2):
                    pair_chunk[g // 2] = (cs, cw * GW, (g - cs) * GW,
                                          g + 2 == cs + cw)

            def emit_tanh_h(P):
                """tanh + h' + (maybe) hn DMA for pair P (c' already done)."""
                cs, cw, lo, last = pair_chunk[P]
                cnb, hnb = cn_hn[P]
                tcp = tp.tile([H, 1024], BF16, name=f"tc{P}", tag="tc")
                nc.scalar.activation(tcp[:], cnb[:, lo:lo + 2 * GW], AF.Tanh)
                sig2 = sig2s.pop(P)
                o3 = sig2[:].rearrange("p (t x) -> p t x", t=2)[:, :, 1024:1536]
                h3 = hnb[:, lo:lo + 2 * GW].rearrange("p (t x) -> p t x", t=2)
                t3 = tcp[:].rearrange("p (t x) -> p t x", t=2)
                nc.vector.tensor_mul(h3, o3, t3)
                if last:
                    nc.sync.dma_start(hnt[:, cs * GW:cs * GW + cw], hnb[:])

            cn_hn = {}
            cn_buf = hn_buf = None
            for P in range(NP):
                g0 = 2 * P
                cs, cw, lo, last = pair_chunk[P]
                if lo == 0:
                    cn_buf = op.tile([H, cw], F16, name=f"cn{g0}", tag="cn")
                    hn_buf = op.tile([H, cw], F16, name=f"hn{g0}", tag="hn")
                cn_hn[P] = (cn_buf, hn_buf)
                sig2 = sp.tile([H, 4096], BF16, name=f"s{P}", tag="sig")
                sig2s[P] = sig2

                def emit_dve(g_first, ng, tag_sfx):
                    """c'-chain for ng groups starting at g_first (pair P).
                    ig/fc/c' are fp16: bf16 rounding of the large ig/fc
                    terms would dominate the error after cancellation."""
                    w = ng * GW
                    gg = g_first - g0

                    def sl(bank):
                        s = sig2[:].rearrange("p (t x) -> p t x", t=2)
                        s = s[:, gg:gg + ng, bank * GW:(bank + 1) * GW]
                        return s

                    def r3(ap2d):
                        return ap2d.rearrange("p (t x) -> p t x", t=ng)

                    c3 = r3(in_slice(cts, g_first, w))
                    gt = tp.tile([H, w], BF16, name=f"gt{tag_sfx}", tag="gt")
                    nc.vector.tensor_scalar(r3(gt[:]), sl(0 + 3), 2.0, 1.0,
                                            ALU.mult, ALU.subtract)
                    ig = tp.tile([H, w], F16, name=f"ig{tag_sfx}", tag="ig")
                    nc.vector.tensor_mul(r3(ig[:]), sl(0), r3(gt[:]))
                    fc = tp.tile([H, w], F16, name=f"fc{tag_sfx}", tag="fc")
                    nc.vector.tensor_mul(r3(fc[:]), sl(1), c3)
                    lg = lo + gg * GW
                    nc.vector.tensor_add(cn_buf[:, lg:lg + w], ig[:], fc[:])
                    if last and gg + ng == 2:
                        nc.sync.dma_start(
                            cnt[:, cs * GW:cs * GW + cw], cn_buf[:])

                lastP = P == NP - 1
                for gg in range(2):
                    g = g0 + gg
                    xs = in_slice(xts, g, GW)
                    hs = in_slice(hts, g, GW)
                    quad = qp.tile([H, 2048], F32, name=f"q{g}", tag="quad")
                    for k in range(4):
                        nc.tensor.matmul(quad[:, k * GW:(k + 1) * GW],
                                         wx_sb[:, k * H:(k + 1) * H], xs,
                                         start=True, stop=False)
                        nc.tensor.matmul(quad[:, k * GW:(k + 1) * GW],
                                         wh_sb[:, k * H:(k + 1) * H], hs,
                                         start=False, stop=True)
                    so = sig2[:, gg * 2048:(gg + 1) * 2048]
                    if has_bias:
                        for k in range(4):
                            nc.scalar.activation(
                                so[:, k * GW:(k + 1) * GW],
                                quad[:, k * GW:(k + 1) * GW],
                                AF.Sigmoid, bias=b_sb[:, k:k + 1])
                    else:
                        nc.scalar.activation(so, quad[:], AF.Sigmoid)
                    if lastP or P == 0:
                        # per-group chain: shortens tail (last pair) and
                        # avoids straddling input chunks (first pair)
                        emit_dve(g, 1, f"p{P}g{gg}")
                    if gg == 1 and P >= 1:
                        emit_tanh_h(P - 1)

                if not (lastP or P == 0):
                    emit_dve(g0, 2, f"p{P}")

            # last pair: per-group tanh/h'/hn to shorten the kernel tail
            P = NP - 1
            cs, cw, lo, _ = pair_chunk[P]
            cnb, hnb = cn_hn[P]
            sig2 = sig2s.pop(P)
            for gg in range(2):
                lg = lo + gg * GW
                tcg = tp.tile([H, GW], BF16, name=f"tcz{gg}", tag="tc")
                nc.scalar.activation(tcg[:], cnb[:, lg:lg + GW], AF.Tanh)
                o2 = sig2[:, gg * 2048 + 1024:gg * 2048 + 1536]
                nc.vector.tensor_mul(hnb[:, lg:lg + GW], o2, tcg[:])
                gcol = (cs + gg * (cw // GW - 1)) * GW
                nc.sync.dma_start(hnt[:, gcol:gcol + GW],
                                  hnb[:, lg:lg + GW])
    nc.compile()
    return nc


def _run(inputs, trace=False, tmpdir=None):
    x = np.asarray(inputs["x"], dtype=np.float32)
    h = np.asarray(inputs["h_t"], dtype=np.float32)
    c = np.asarray(inputs["c_t"], dtype=np.float32)
    # gate order [i, f, o, g]; W_g/b_g scaled by 2 for the tanh-via-sigmoid
    wx = np.concatenate([inputs["W_ii"], inputs["W_if"], inputs["W_io"],
                         2.0 * np.asarray(inputs["W_ig"])], axis=0)
    wh = np.concatenate([inputs["W_hi"], inputs["W_hf"], inputs["W_ho"],
                         2.0 * np.asarray(inputs["W_hg"])], axis=0)
    b = np.concatenate([inputs["b_i"], inputs["b_f"], inputs["b_o"],
                        2.0 * np.asarray(inputs["b_g"])], axis=0)
    wxt = np.ascontiguousarray(wx.T).astype(np.float16)
    wht = np.ascontiguousarray(wh.T).astype(np.float16)
    has_bias = bool(np.any(b))

    key = has_bias
    if key not in _CACHE:
        _CACHE[key] = _build(has_bias)
    nc = _CACHE[key]

    x16 = x.astype(np.float16)
    h16 = h.astype(np.float16)
    c16 = c.astype(np.float16)
    in_maps = []
    for i in range(NCORES):
        s = slice(i * BC, (i + 1) * BC)
        m = {
            "xt": np.ascontiguousarray(x16[s].T),
            "ht": np.ascontiguousarray(h16[s].T),
            "ct": np.ascontiguousarray(c16[s].T),
            "wxt": wxt,
            "wht": wht,
        }
        if has_bias:
            m["bias"] = np.ascontiguousarray(
                b.reshape(4, H).T.astype(np.float32))
        in_maps.append(m)

    res = run_bass_kernel_spmd(nc, in_maps, core_ids=list(range(NCORES)),
                               trace=trace, tmpdir=tmpdir)
    h_new = np.empty((NCORES * BC, H), dtype=np.float32)
    c_new = np.empty((NCORES * BC, H), dtype=np.float32)
    for i, r in enumerate(res.results):
        s = slice(i * BC, (i + 1) * BC)
        h_new[s] = r["hnt"].T
        c_new[s] = r["cnt"].T
    return h_new, c_new, res


def kernel(**inputs):
    h_new, c_new, _ = _run(inputs, trace=False)
    return h_new, c_new


# revision 24
# speedup vs baseline: 1.5372x; 1.0008x over previous
"""LSTMCell on 8 Trainium2 NeuronCores, data-parallel over the batch.

Full inputs: x/h_t/c_t [65536,128] f32, 8 gate weight matrices [128,128],
4 biases [128]. Returns (h_new, c_new) as [65536,128] f32 each.

v3 design (all-bf16, transposed layout, zero on-device transposes):
  - Host transposes x/h/c per core to xT/hT/cT [128 feat, 8192 batch] bf16
    and pre-concats weights as WxT/WhT [128 in, 512 gate-rows] bf16 in gate
    order [i, f, o, 2*g] (g prescaled by 2 for the tanh-via-sigmoid trick).
  - Per batch group of 512 columns: 8 bf16 matmuls (weights stationary,
    batch streams; PE issue period ~259ns) accumulate gates^T into a 4-bank
    PSUM quad [128, 2048]: banks = i|f|o|2g pre-acts.
  - ONE sigmoid per quad -> bf16 SBUF (bf16 ACT output is ~1.7x faster than
    fp16 out); two groups share a sig2 tile [128, 4096] so DVE ops batch
    per PAIR of groups via 3D APs (keeps the DVE 2x bf16 mode).
  - DVE per pair: gt=2s-1 [TS 4x], ig=i*gt [TT], fc=f*c [TT],
    c'=ig+fc [TT], h'=o*tanh(c') [TT].
  - tanh(c') on ACT, delayed one pair so ACT never stalls on the DVE chain.
  - DMA: inputs split 4+12 groups (fast fill + big descriptors), outputs
    per 8 groups (1MB, 8KB/partition descriptors).
"""
import numpy as np
import ml_dtypes
from contextlib import ExitStack

import concourse.bass as bass
import concourse.tile as tile
from concourse import bacc, mybir
from concourse.bass_utils import run_bass_kernel_spmd

F32 = mybir.dt.float32
F16 = mybir.dt.float16
BF16 = mybir.dt.bfloat16
NPBF = ml_dtypes.bfloat16
AF = mybir.ActivationFunctionType
ALU = mybir.AluOpType

NCORES = 8
BC = 8192            # batch rows per core
GW = 512             # batch columns per group (one PSUM bank)
NG = BC // GW        # 16 groups
H = 128              # hidden size
# input chunks in groups: small (fast fill), then growing
ICHUNKS = [(0, 1), (1, 1), (2, 2), (4, 4), (8, 8)]
# output chunks (start group, n groups): big, medium, small tail
OCHUNKS = [(0, 8), (8, 6), (14, 2)]

_CACHE = {}


def _build(has_bias: bool):
    nc = bacc.Bacc("TRN2", target_bir_lowering=False, debug=False)
    xt = nc.dram_tensor("xt", [H, BC], F16, kind="ExternalInput").ap()
    ht = nc.dram_tensor("ht", [H, BC], F16, kind="ExternalInput").ap()
    ct = nc.dram_tensor("ct", [H, BC], F16, kind="ExternalInput").ap()
    wxt = nc.dram_tensor("wxt", [H, 4 * H], F16, kind="ExternalInput").ap()
    wht = nc.dram_tensor("wht", [H, 4 * H], F16, kind="ExternalInput").ap()
    if has_bias:
        bias = nc.dram_tensor("bias", [H, 4], F32, kind="ExternalInput").ap()
    hnt = nc.dram_tensor("hnt", [H, BC], F16, kind="ExternalOutput").ap()
    cnt = nc.dram_tensor("cnt", [H, BC], F16, kind="ExternalOutput").ap()



    with tile.TileContext(nc) as tc:
        with ExitStack() as ctx:
            const = ctx.enter_context(tc.tile_pool(name="const", bufs=1))
            ina = ctx.enter_context(tc.tile_pool(name="ina", bufs=1))
            qp = ctx.enter_context(tc.tile_pool(name="qp", bufs=2, space="PSUM"))
            sp = ctx.enter_context(tc.tile_pool(name="sp", bufs=3))
            tp = ctx.enter_context(tc.tile_pool(name="tp", bufs=2))
            op = ctx.enter_context(tc.tile_pool(name="op", bufs=2))

            # Input tiles in 3 chunks per tensor: small chunk first for fast
            # pipeline fill, then medium/large for DMA efficiency.  x/h
            # chunks issue before c (c is consumed later, by the DVE chain).
            xts, hts, cts = [], [], []
            for ci, (cs, cw) in enumerate(ICHUNKS):
                for lst, nm in ((xts, "x"), (hts, "h"), (cts, "c")):
                    lst.append(ina.tile([H, cw * GW], F16,
                                        name=f"{nm}{ci}"))
            def cbounds(ci):
                cs, cw = ICHUNKS[ci]
                return cs * GW, (cs + cw) * GW
            for ci in range(len(ICHUNKS)):
                c0, c1 = cbounds(ci)
                nc.sync.dma_start(xts[ci][:], xt[:, c0:c1])
                nc.sync.dma_start(hts[ci][:], ht[:, c0:c1])
                if ci == 0:
                    wx_sb = const.tile([H, 4 * H], F16)
                    nc.sync.dma_start(wx_sb[:], wxt)
                    wh_sb = const.tile([H, 4 * H], F16)
                    nc.sync.dma_start(wh_sb[:], wht)
                    if has_bias:
                        b_sb = const.tile([H, 4], F32)
                        nc.sync.dma_start(b_sb[:], bias)
                else:
                    c0p, c1p = cbounds(ci - 1)
                    nc.sync.dma_start(cts[ci - 1][:], ct[:, c0p:c1p])
            c0p, c1p = cbounds(len(ICHUNKS) - 1)
            nc.sync.dma_start(cts[-1][:], ct[:, c0p:c1p])

            # ACT table preload (sigmoid set includes tanh) overlaps DMA fill
            dummy = const.tile([H, 8], F32)
            nc.vector.memset(dummy[:], 0.0)
            dummy2 = const.tile([H, 8], F32)
            nc.scalar.activation(dummy2[:], dummy[:], AF.Sigmoid)

            def in_slice(tiles, g, w):
                c0 = g * GW
                for ci, (cs, cw) in enumerate(ICHUNKS):
                    if c0 + w <= (cs + cw) * GW:
                        return tiles[ci][:, c0 - cs * GW:c0 - cs * GW + w]
                raise AssertionError("slice straddles input chunks")

            # HAM warmup on a junk tile while DMAs stream
            junk = const.tile([H, GW], F16)
            nc.vector.memset(junk[:], 0.0)
            warm = qp.tile([H, 2048], F32, name="warm", tag="quad")
            for _ in range(5):
                nc.tensor.matmul(warm[:, 0:GW], junk[:, 0:H], junk[:],
                                 start=True, stop=True)

            NP = NG // 2  # pairs
            sig2s = {}

            # pair -> (chunk_start_group, chunk_width, local_offset, is_last)
            pair_chunk = {}
            for cs, cw in OCHUNKS:
                for g in range(cs, cs + cw, 2):
                    pair_chunk[g // 2] = (cs, cw * GW, (g - cs) * GW,
                                          g + 2 == cs + cw)

            def emit_tanh_h(P):
                """tanh + h' + (maybe) hn DMA for pair P (c' already done)."""
                cs, cw, lo, last = pair_chunk[P]
                cnb, hnb = cn_hn[P]
                tcp = tp.tile([H, 1024], BF16, name=f"tc{P}", tag="tc")
                nc.scalar.activation(tcp[:], cnb[:, lo:lo + 2 * GW], AF.Tanh)
                sig2 = sig2s.pop(P)
                o3 = sig2[:].rearrange("p (t x) -> p t x", t=2)[:, :, 1024:1536]
                h3 = hnb[:, lo:lo + 2 * GW].rearrange("p (t x) -> p t x", t=2)
                t3 = tcp[:].rearrange("p (t x) -> p t x", t=2)
                nc.vector.tensor_mul(h3, o3, t3)
                if last:
                    nc.sync.dma_start(hnt[:, cs * GW:cs * GW + cw], hnb[:])

            cn_hn = {}
            cn_buf = hn_buf = None
            for P in range(NP):
                g0 = 2 * P
                cs, cw, lo, last = pair_chunk[P]
                if lo == 0:
                    cn_buf = op.tile([H, cw], F16, name=f"cn{g0}", tag="cn")
                    hn_buf = op.tile([H, cw], F16, name=f"hn{g0}", tag="hn")
                cn_hn[P] = (cn_buf, hn_buf)
                sig2 = sp.tile([H, 4096], BF16, name=f"s{P}", tag="sig")
                sig2s[P] = sig2

                def emit_dve(g_first, ng, tag_sfx):
                    """c'-chain for ng groups starting at g_first (pair P).
                    ig/fc/c' are fp16: bf16 rounding of the large ig/fc
                    terms would dominate the error after cancellation."""
                    w = ng * GW
                    gg = g_first - g0

                    def sl(bank):
                        s = sig2[:].rearrange("p (t x) -> p t x", t=2)
                        s = s[:, gg:gg + ng, bank * GW:(bank + 1) * GW]
                        return s

                    def r3(ap2d):
                        return ap2d.rearrange("p (t x) -> p t x", t=ng)

                    c3 = r3(in_slice(cts, g_first, w))
                    gt = tp.tile([H, w], BF16, name=f"gt{tag_sfx}", tag="gt")
                    nc.vector.tensor_scalar(r3(gt[:]), sl(0 + 3), 2.0, 1.0,
                                            ALU.mult, ALU.subtract)
                    ig = tp.tile([H, w], F16, name=f"ig{tag_sfx}", tag="ig")
                    nc.vector.tensor_mul(r3(ig[:]), sl(0), r3(gt[:]))
                    fc = tp.tile([H, w], F16, name=f"fc{tag_sfx}", tag="fc")
                    nc.vector.tensor_mul(r3(fc[:]), sl(1), c3)
                    lg = lo + gg * GW
                    nc.vector.tensor_add(cn_buf[:, lg:lg + w], ig[:], fc[:])
                    if last and gg + ng == 2:
                        nc.sync.dma_start(
                            cnt[:, cs * GW:cs * GW + cw], cn_buf[:])

                lastP = P == NP - 1
                for gg in range(2):
                    g = g0 + gg
                    xs = in_slice(xts, g, GW)
                    hs = in_slice(hts, g, GW)
                    halves = 2 if (P == 0 and gg == 0 and not has_bias) else 1
                    quad = qp.tile([H, 2048], F32, name=f"q{g}", tag="quad")
                    so = sig2[:, gg * 2048:(gg + 1) * 2048]
                    for half in range(halves):
                        k0, k1 = (0, 4) if halves == 1 else                             (2 * half, 2 * half + 2)
                        for k in range(k0, k1):
                            nc.tensor.matmul(quad[:, k * GW:(k + 1) * GW],
                                             wx_sb[:, k * H:(k + 1) * H], xs,
                                             start=True, stop=False)
                            nc.tensor.matmul(quad[:, k * GW:(k + 1) * GW],
                                             wh_sb[:, k * H:(k + 1) * H], hs,
                                             start=False, stop=True)
                        if has_bias:
                            for k in range(4):
                                nc.scalar.activation(
                                    so[:, k * GW:(k + 1) * GW],
                                    quad[:, k * GW:(k + 1) * GW],
                                    AF.Sigmoid, bias=b_sb[:, k:k + 1])
                        else:
                            c0h, c1h = k0 * GW, k1 * GW
                            nc.scalar.activation(so[:, c0h:c1h],
                                                 quad[:, c0h:c1h], AF.Sigmoid)
                    if lastP or P == 0:
                        # per-group chain: shortens tail (last pair) and
                        # avoids straddling input chunks (first pair)
                        emit_dve(g, 1, f"p{P}g{gg}")
                    if gg == 1 and P >= 1:
                        emit_tanh_h(P - 1)

                if not (lastP or P == 0):
                    emit_dve(g0, 2, f"p{P}")

            # last pair: per-group tanh/h'/hn to shorten the kernel tail
            P = NP - 1
            cs, cw, lo, _ = pair_chunk[P]
            cnb, hnb = cn_hn[P]
            sig2 = sig2s.pop(P)
            for gg in range(2):
                lg = lo + gg * GW
                tcg = tp.tile([H, GW], BF16, name=f"tcz{gg}", tag="tc")
                nc.scalar.activation(tcg[:], cnb[:, lg:lg + GW], AF.Tanh)
                o2 = sig2[:, gg * 2048 + 1024:gg * 2048 + 1536]
                nc.vector.tensor_mul(hnb[:, lg:lg + GW], o2, tcg[:])
                gcol = (cs + gg * (cw // GW - 1)) * GW
                nc.sync.dma_start(hnt[:, gcol:gcol + GW],
                                  hnb[:, lg:lg + GW])
    nc.compile()
    return nc


def _run(inputs, trace=False, tmpdir=None):
    x = np.asarray(inputs["x"], dtype=np.float32)
    h = np.asarray(inputs["h_t"], dtype=np.float32)
    c = np.asarray(inputs["c_t"], dtype=np.float32)
    # gate order [i, f, o, g]; W_g/b_g scaled by 2 for the tanh-via-sigmoid
    wx = np.concatenate([inputs["W_ii"], inputs["W_if"], inputs["W_io"],
                         2.0 * np.asarray(inputs["W_ig"])], axis=0)
    wh = np.concatenate([inputs["W_hi"], inputs["W_hf"], inputs["W_ho"],
                         2.0 * np.asarray(inputs["W_hg"])], axis=0)
    b = np.concatenate([inputs["b_i"], inputs["b_f"], inputs["b_o"],
                        2.0 * np.asarray(inputs["b_g"])], axis=0)
    wxt = np.ascontiguousarray(wx.T).astype(np.float16)
    wht = np.ascontiguousarray(wh.T).astype(np.float16)
    has_bias = bool(np.any(b))

    key = has_bias
    if key not in _CACHE:
        _CACHE[key] = _build(has_bias)
    nc = _CACHE[key]

    x16 = x.astype(np.float16)
    h16 = h.astype(np.float16)
    c16 = c.astype(np.float16)
    in_maps = []
    for i in range(NCORES):
        s = slice(i * BC, (i + 1) * BC)
        m = {
            "xt": np.ascontiguousarray(x16[s].T),
            "ht": np.ascontiguousarray(h16[s].T),
            "ct": np.ascontiguousarray(c16[s].T),
            "wxt": wxt,
            "wht": wht,
        }
        if has_bias:
            m["bias"] = np.ascontiguousarray(
                b.reshape(4, H).T.astype(np.float32))
        in_maps.append(m)

    res = run_bass_kernel_spmd(nc, in_maps, core_ids=list(range(NCORES)),
                               trace=trace, tmpdir=tmpdir)
    h_new = np.empty((NCORES * BC, H), dtype=np.float32)
    c_new = np.empty((NCORES * BC, H), dtype=np.float32)
    for i, r in enumerate(res.results):
        s = slice(i * BC, (i + 1) * BC)
        h_new[s] = r["hnt"].T
        c_new[s] = r["cnt"].T
    return h_new, c_new, res


def kernel(**inputs):
    h_new, c_new, _ = _run(inputs, trace=False)
    return h_new, c_new


# revision 25
# speedup vs baseline: 1.5394x; 1.0014x over previous
"""LSTMCell on 8 Trainium2 NeuronCores, data-parallel over the batch.

Full inputs: x/h_t/c_t [65536,128] f32, 8 gate weight matrices [128,128],
4 biases [128]. Returns (h_new, c_new) as [65536,128] f32 each.

v3 design (all-bf16, transposed layout, zero on-device transposes):
  - Host transposes x/h/c per core to xT/hT/cT [128 feat, 8192 batch] bf16
    and pre-concats weights as WxT/WhT [128 in, 512 gate-rows] bf16 in gate
    order [i, f, o, 2*g] (g prescaled by 2 for the tanh-via-sigmoid trick).
  - Per batch group of 512 columns: 8 bf16 matmuls (weights stationary,
    batch streams; PE issue period ~259ns) accumulate gates^T into a 4-bank
    PSUM quad [128, 2048]: banks = i|f|o|2g pre-acts.
  - ONE sigmoid per quad -> bf16 SBUF (bf16 ACT output is ~1.7x faster than
    fp16 out); two groups share a sig2 tile [128, 4096] so DVE ops batch
    per PAIR of groups via 3D APs (keeps the DVE 2x bf16 mode).
  - DVE per pair: gt=2s-1 [TS 4x], ig=i*gt [TT], fc=f*c [TT],
    c'=ig+fc [TT], h'=o*tanh(c') [TT].
  - tanh(c') on ACT, delayed one pair so ACT never stalls on the DVE chain.
  - DMA: inputs split 4+12 groups (fast fill + big descriptors), outputs
    per 8 groups (1MB, 8KB/partition descriptors).
"""
import numpy as np
import ml_dtypes
from contextlib import ExitStack

import concourse.bass as bass
import concourse.tile as tile
from concourse import bacc, mybir
from concourse.bass_utils import run_bass_kernel_spmd

F32 = mybir.dt.float32
F16 = mybir.dt.float16
BF16 = mybir.dt.bfloat16
NPBF = ml_dtypes.bfloat16
AF = mybir.ActivationFunctionType
ALU = mybir.AluOpType

NCORES = 8
BC = 8192            # batch rows per core
GW = 512             # batch columns per group (one PSUM bank)
NG = BC // GW        # 16 groups
H = 128              # hidden size
# input chunks in groups: small (fast fill), then growing
ICHUNKS = [(0, 1), (1, 1), (2, 2), (4, 4), (8, 8)]
# output chunks (start group, n groups): big, medium, small tail
OCHUNKS = [(0, 8), (8, 6), (14, 2)]

_CACHE = {}


def _build(has_bias: bool):
    nc = bacc.Bacc("TRN2", target_bir_lowering=False, debug=False)
    xt = nc.dram_tensor("xt", [H, BC], F16, kind="ExternalInput").ap()
    ht = nc.dram_tensor("ht", [H, BC], F16, kind="ExternalInput").ap()
    ct = nc.dram_tensor("ct", [H, BC], F16, kind="ExternalInput").ap()
    wxt = nc.dram_tensor("wxt", [H, 4 * H], F16, kind="ExternalInput").ap()
    wht = nc.dram_tensor("wht", [H, 4 * H], F16, kind="ExternalInput").ap()
    if has_bias:
        bias = nc.dram_tensor("bias", [H, 4], F32, kind="ExternalInput").ap()
    hnt = nc.dram_tensor("hnt", [H, BC], F16, kind="ExternalOutput").ap()
    cnt = nc.dram_tensor("cnt", [H, BC], F16, kind="ExternalOutput").ap()



    with tile.TileContext(nc) as tc:
        with ExitStack() as ctx:
            const = ctx.enter_context(tc.tile_pool(name="const", bufs=1))
            ina = ctx.enter_context(tc.tile_pool(name="ina", bufs=1))
            qp = ctx.enter_context(tc.tile_pool(name="qp", bufs=2, space="PSUM"))
            sp = ctx.enter_context(tc.tile_pool(name="sp", bufs=4))
            tp = ctx.enter_context(tc.tile_pool(name="tp", bufs=2))
            op = ctx.enter_context(tc.tile_pool(name="op", bufs=2))

            # Input tiles in 3 chunks per tensor: small chunk first for fast
            # pipeline fill, then medium/large for DMA efficiency.  x/h
            # chunks issue before c (c is consumed later, by the DVE chain).
            xts, hts, cts = [], [], []
            for ci, (cs, cw) in enumerate(ICHUNKS):
                for lst, nm in ((xts, "x"), (hts, "h"), (cts, "c")):
                    lst.append(ina.tile([H, cw * GW], F16,
                                        name=f"{nm}{ci}"))
            def cbounds(ci):
                cs, cw = ICHUNKS[ci]
                return cs * GW, (cs + cw) * GW
            for ci in range(len(ICHUNKS)):
                c0, c1 = cbounds(ci)
                nc.sync.dma_start(xts[ci][:], xt[:, c0:c1])
                nc.sync.dma_start(hts[ci][:], ht[:, c0:c1])
                if ci == 0:
                    wx_sb = const.tile([H, 4 * H], F16)
                    nc.sync.dma_start(wx_sb[:], wxt)
                    wh_sb = const.tile([H, 4 * H], F16)
                    nc.sync.dma_start(wh_sb[:], wht)
                    if has_bias:
                        b_sb = const.tile([H, 4], F32)
                        nc.sync.dma_start(b_sb[:], bias)
                else:
                    c0p, c1p = cbounds(ci - 1)
                    nc.sync.dma_start(cts[ci - 1][:], ct[:, c0p:c1p])
            c0p, c1p = cbounds(len(ICHUNKS) - 1)
            nc.sync.dma_start(cts[-1][:], ct[:, c0p:c1p])

            # ACT table preload (sigmoid set includes tanh) overlaps DMA fill
            dummy = const.tile([H, 8], F32)
            nc.vector.memset(dummy[:], 0.0)
            dummy2 = const.tile([H, 8], F32)
            nc.scalar.activation(dummy2[:], dummy[:], AF.Sigmoid)

            def in_slice(tiles, g, w):
                c0 = g * GW
                for ci, (cs, cw) in enumerate(ICHUNKS):
                    if c0 + w <= (cs + cw) * GW:
                        return tiles[ci][:, c0 - cs * GW:c0 - cs * GW + w]
                raise AssertionError("slice straddles input chunks")

            # HAM warmup on a junk tile while DMAs stream
            junk = const.tile([H, GW], F16)
            nc.vector.memset(junk[:], 0.0)
            warm = qp.tile([H, 2048], F32, name="warm", tag="quad")
            for _ in range(5):
                nc.tensor.matmul(warm[:, 0:GW], junk[:, 0:H], junk[:],
                                 start=True, stop=True)

            NP = NG // 2  # pairs
            sig2s = {}

            # pair -> (chunk_start_group, chunk_width, local_offset, is_last)
            pair_chunk = {}
            for cs, cw in OCHUNKS:
                for g in range(cs, cs + cw, 2):
                    pair_chunk[g // 2] = (cs, cw * GW, (g - cs) * GW,
                                          g + 2 == cs + cw)

            def emit_tanh_h2(Pa):
                """tanh + h' for pairs (Pa, Pa+1) in one ACT pass."""
                Pb = Pa + 1
                cs, cw, lo_a, _ = pair_chunk[Pa]
                cnb, hnb = cn_hn[Pa]
                tcp = tp.tile([H, 2048], BF16, name=f"tc{Pa}", tag="tc")
                nc.scalar.activation(tcp[:], cnb[:, lo_a:lo_a + 4 * GW],
                                     AF.Tanh)
                for j, P in enumerate((Pa, Pb)):
                    lo = pair_chunk[P][2]
                    last = pair_chunk[P][3]
                    sig2 = sig2s.pop(P)
                    o3 = sig2[:].rearrange("p (t x) -> p t x",
                                           t=2)[:, :, 1024:1536]
                    h3 = hnb[:, lo:lo + 2 * GW].rearrange(
                        "p (t x) -> p t x", t=2)
                    t3 = tcp[:, j * 1024:(j + 1) * 1024].rearrange(
                        "p (t x) -> p t x", t=2)
                    nc.vector.tensor_mul(h3, o3, t3)
                    if last:
                        nc.sync.dma_start(hnt[:, cs * GW:cs * GW + cw],
                                          hnb[:])

            def emit_tanh_h(P):
                """tanh + h' + (maybe) hn DMA for pair P (c' already done)."""
                cs, cw, lo, last = pair_chunk[P]
                cnb, hnb = cn_hn[P]
                tcp = tp.tile([H, 1024], BF16, name=f"tc{P}", tag="tc")
                nc.scalar.activation(tcp[:], cnb[:, lo:lo + 2 * GW], AF.Tanh)
                sig2 = sig2s.pop(P)
                o3 = sig2[:].rearrange("p (t x) -> p t x", t=2)[:, :, 1024:1536]
                h3 = hnb[:, lo:lo + 2 * GW].rearrange("p (t x) -> p t x", t=2)
                t3 = tcp[:].rearrange("p (t x) -> p t x", t=2)
                nc.vector.tensor_mul(h3, o3, t3)
                if last:
                    nc.sync.dma_start(hnt[:, cs * GW:cs * GW + cw], hnb[:])

            cn_hn = {}
            cn_buf = hn_buf = None
            for P in range(NP):
                g0 = 2 * P
                cs, cw, lo, last = pair_chunk[P]
                if lo == 0:
                    cn_buf = op.tile([H, cw], F16, name=f"cn{g0}", tag="cn")
                    hn_buf = op.tile([H, cw], F16, name=f"hn{g0}", tag="hn")
                cn_hn[P] = (cn_buf, hn_buf)
                sig2 = sp.tile([H, 4096], BF16, name=f"s{P}", tag="sig")
                sig2s[P] = sig2

                def emit_dve(g_first, ng, tag_sfx):
                    """c'-chain for ng groups starting at g_first (pair P).
                    ig/fc/c' are fp16: bf16 rounding of the large ig/fc
                    terms would dominate the error after cancellation."""
                    w = ng * GW
                    gg = g_first - g0

                    def sl(bank):
                        s = sig2[:].rearrange("p (t x) -> p t x", t=2)
                        s = s[:, gg:gg + ng, bank * GW:(bank + 1) * GW]
                        return s

                    def r3(ap2d):
                        return ap2d.rearrange("p (t x) -> p t x", t=ng)

                    c3 = r3(in_slice(cts, g_first, w))
                    gt = tp.tile([H, w], BF16, name=f"gt{tag_sfx}", tag="gt")
                    nc.vector.tensor_scalar(r3(gt[:]), sl(0 + 3), 2.0, 1.0,
                                            ALU.mult, ALU.subtract)
                    ig = tp.tile([H, w], F16, name=f"ig{tag_sfx}", tag="ig")
                    nc.vector.tensor_mul(r3(ig[:]), sl(0), r3(gt[:]))
                    fc = tp.tile([H, w], F16, name=f"fc{tag_sfx}", tag="fc")
                    nc.vector.tensor_mul(r3(fc[:]), sl(1), c3)
                    lg = lo + gg * GW
                    nc.vector.tensor_add(cn_buf[:, lg:lg + w], ig[:], fc[:])
                    if last and gg + ng == 2:
                        nc.sync.dma_start(
                            cnt[:, cs * GW:cs * GW + cw], cn_buf[:])

                lastP = P == NP - 1
                for gg in range(2):
                    g = g0 + gg
                    xs = in_slice(xts, g, GW)
                    hs = in_slice(hts, g, GW)
                    halves = 2 if (P == 0 and gg == 0 and not has_bias) else 1
                    quad = qp.tile([H, 2048], F32, name=f"q{g}", tag="quad")
                    so = sig2[:, gg * 2048:(gg + 1) * 2048]
                    for half in range(halves):
                        k0, k1 = (0, 4) if halves == 1 else                             (2 * half, 2 * half + 2)
                        for k in range(k0, k1):
                            nc.tensor.matmul(quad[:, k * GW:(k + 1) * GW],
                                             wx_sb[:, k * H:(k + 1) * H], xs,
                                             start=True, stop=False)
                            nc.tensor.matmul(quad[:, k * GW:(k + 1) * GW],
                                             wh_sb[:, k * H:(k + 1) * H], hs,
                                             start=False, stop=True)
                        if has_bias:
                            for k in range(4):
                                nc.scalar.activation(
                                    so[:, k * GW:(k + 1) * GW],
                                    quad[:, k * GW:(k + 1) * GW],
                                    AF.Sigmoid, bias=b_sb[:, k:k + 1])
                        else:
                            c0h, c1h = k0 * GW, k1 * GW
                            nc.scalar.activation(so[:, c0h:c1h],
                                                 quad[:, c0h:c1h], AF.Sigmoid)
                    if lastP or P == 0:
                        # per-group chain: shortens tail (last pair) and
                        # avoids straddling input chunks (first pair)
                        emit_dve(g, 1, f"p{P}g{gg}")
                    if gg == 1:
                        if P in (2, 4, 6):
                            emit_tanh_h2(P - 2)
                        elif P == 7:
                            emit_tanh_h(P - 1)

                if not (lastP or P == 0):
                    emit_dve(g0, 2, f"p{P}")

            # last pair: per-group tanh/h'/hn to shorten the kernel tail
            P = NP - 1
            cs, cw, lo, _ = pair_chunk[P]
            cnb, hnb = cn_hn[P]
            sig2 = sig2s.pop(P)
            for gg in range(2):
                lg = lo + gg * GW
                tcg = tp.tile([H, GW], BF16, name=f"tcz{gg}", tag="tc")
                nc.scalar.activation(tcg[:], cnb[:, lg:lg + GW], AF.Tanh)
                o2 = sig2[:, gg * 2048 + 1024:gg * 2048 + 1536]
                nc.vector.tensor_mul(hnb[:, lg:lg + GW], o2, tcg[:])
                gcol = (cs + gg * (cw // GW - 1)) * GW
                nc.sync.dma_start(hnt[:, gcol:gcol + GW],
                                  hnb[:, lg:lg + GW])
    nc.compile()
    return nc


def _run(inputs, trace=False, tmpdir=None):
    x = np.asarray(inputs["x"], dtype=np.float32)
    h = np.asarray(inputs["h_t"], dtype=np.float32)
    c = np.asarray(inputs["c_t"], dtype=np.float32)
    # gate order [i, f, o, g]; W_g/b_g scaled by 2 for the tanh-via-sigmoid
    wx = np.concatenate([inputs["W_ii"], inputs["W_if"], inputs["W_io"],
                         2.0 * np.asarray(inputs["W_ig"])], axis=0)
    wh = np.concatenate([inputs["W_hi"], inputs["W_hf"], inputs["W_ho"],
                         2.0 * np.asarray(inputs["W_hg"])], axis=0)
    b = np.concatenate([inputs["b_i"], inputs["b_f"], inputs["b_o"],
                        2.0 * np.asarray(inputs["b_g"])], axis=0)
    wxt = np.ascontiguousarray(wx.T).astype(np.float16)
    wht = np.ascontiguousarray(wh.T).astype(np.float16)
    has_bias = bool(np.any(b))

    key = has_bias
    if key not in _CACHE:
        _CACHE[key] = _build(has_bias)
    nc = _CACHE[key]

    x16 = x.astype(np.float16)
    h16 = h.astype(np.float16)
    c16 = c.astype(np.float16)
    in_maps = []
    for i in range(NCORES):
        s = slice(i * BC, (i + 1) * BC)
        m = {
            "xt": np.ascontiguousarray(x16[s].T),
            "ht": np.ascontiguousarray(h16[s].T),
            "ct": np.ascontiguousarray(c16[s].T),
            "wxt": wxt,
            "wht": wht,
        }
        if has_bias:
            m["bias"] = np.ascontiguousarray(
                b.reshape(4, H).T.astype(np.float32))
        in_maps.append(m)

    res = run_bass_kernel_spmd(nc, in_maps, core_ids=list(range(NCORES)),
                               trace=trace, tmpdir=tmpdir)
    h_new = np.empty((NCORES * BC, H), dtype=np.float32)
    c_new = np.empty((NCORES * BC, H), dtype=np.float32)
    for i, r in enumerate(res.results):
        s = slice(i * BC, (i + 1) * BC)
        h_new[s] = r["hnt"].T
        c_new[s] = r["cnt"].T
    return h_new, c_new, res


def kernel(**inputs):
    h_new, c_new, _ = _run(inputs, trace=False)
    return h_new, c_new


# revision 26
# speedup vs baseline: 1.5524x; 1.0085x over previous
"""LSTMCell on 8 Trainium2 NeuronCores, data-parallel over the batch.

Full inputs: x/h_t/c_t [65536,128] f32, 8 gate weight matrices [128,128],
4 biases [128]. Returns (h_new, c_new) as [65536,128] f32 each.

Design (v11, ~61us; fp16 matmul path, transposed layout, no on-device
transposes; steady state is ACT(sigmoid)-bound):
  - Host transposes x/h/c per core to [128 feat, 8192 batch] fp16 and
    pre-concats weights as WxT/WhT [128 in, 512 gate-rows] fp16 in gate
    order [i, f, o, 2*g] (g prescaled by 2 for the tanh-via-sigmoid trick).
    fp16 (not bf16) operands: the bf16 rounding of x/h/W through the gates
    was the dominant error term (1.2e-2); fp16 cuts it ~8x at zero cost
    (PE streams fp16 == bf16: ~216-260ns issue period per N=512 matmul).
  - Per batch group of 512 cols: 8 matmuls (weights stationary) accumulate
    gates^T into a 4-bank PSUM quad [128, 2048] = i|f|o|2g pre-acts.
  - ONE sigmoid per quad -> bf16 SBUF (ACT 16-bit-out runs ~0.9ns/elem;
    bf16 out is fastest of the 16-bit options; f32-out would be 2x faster
    on ACT but forces the DVE chain to 1x mode = net loss). Two groups
    share a sig2 tile [128, 4096] so DVE ops batch per PAIR via 3D APs
    (2-byte dtypes keep the DVE 2x mode, ~0.67ns/elem).
  - DVE per pair: gt=2s-1 [TS], ig=i*gt, fc=f*c, c'=ig+fc, h'=o*tanh(c').
    ig/fc/c'/h' and the c input are fp16: bf16 rounding of the large ig/fc
    terms dominated the error after cancellation in c'.
  - tanh(c') on ACT, delayed one pair (emitted after the next pair's
    sigmoids) so ACT never stalls on the DVE chain; batched across 2 pairs
    mid-pipeline; per-group at the tail to shorten the critical path.
  - DMA: inputs in 5 chunks/tensor (1,1,2,4,8 groups - fast pipeline fill,
    then big descriptors; 4KB-contig descriptors cap the HWDGE ring at
    ~258GB/s so later chunks use 2-8KB lines), x/h ahead of c; outputs in
    8/6/2-group chunks with per-group DMAs at the very end.
"""
import numpy as np
import ml_dtypes
from contextlib import ExitStack

import concourse.bass as bass
import concourse.tile as tile
from concourse import bacc, mybir
from concourse.bass_utils import run_bass_kernel_spmd

F32 = mybir.dt.float32
F16 = mybir.dt.float16
BF16 = mybir.dt.bfloat16
NPBF = ml_dtypes.bfloat16
AF = mybir.ActivationFunctionType
ALU = mybir.AluOpType

NCORES = 8
BC = 8192            # batch rows per core
GW = 512             # batch columns per group (one PSUM bank)
NG = BC // GW        # 16 groups
H = 128              # hidden size
# input chunks in groups: small (fast fill), then growing
ICHUNKS = [(0, 1), (1, 1), (2, 2), (4, 4), (8, 8)]
# output chunks (start group, n groups): big, medium, small tail
OCHUNKS = [(0, 8), (8, 6), (14, 2)]

_CACHE = {}


def _build(has_bias: bool):
    nc = bacc.Bacc("TRN2", target_bir_lowering=False, debug=False)
    xt = nc.dram_tensor("xt", [H, BC], F16, kind="ExternalInput").ap()
    ht = nc.dram_tensor("ht", [H, BC], F16, kind="ExternalInput").ap()
    ct = nc.dram_tensor("ct", [H, BC], F16, kind="ExternalInput").ap()
    wxt = nc.dram_tensor("wxt", [H, 4 * H], F16, kind="ExternalInput").ap()
    wht = nc.dram_tensor("wht", [H, 4 * H], F16, kind="ExternalInput").ap()
    if has_bias:
        bias = nc.dram_tensor("bias", [H, 4], F32, kind="ExternalInput").ap()
    hnt = nc.dram_tensor("hnt", [H, BC], F16, kind="ExternalOutput").ap()
    cnt = nc.dram_tensor("cnt", [H, BC], F16, kind="ExternalOutput").ap()



    with tile.TileContext(nc) as tc:
        with ExitStack() as ctx:
            const = ctx.enter_context(tc.tile_pool(name="const", bufs=1))
            ina = ctx.enter_context(tc.tile_pool(name="ina", bufs=1))
            qp = ctx.enter_context(tc.tile_pool(name="qp", bufs=2, space="PSUM"))
            sp = ctx.enter_context(tc.tile_pool(name="sp", bufs=4))
            tp = ctx.enter_context(tc.tile_pool(name="tp", bufs=2))
            op = ctx.enter_context(tc.tile_pool(name="op", bufs=2))

            # Input tiles in 3 chunks per tensor: small chunk first for fast
            # pipeline fill, then medium/large for DMA efficiency.  x/h
            # chunks issue before c (c is consumed later, by the DVE chain).
            xts, hts, cts = [], [], []
            for ci, (cs, cw) in enumerate(ICHUNKS):
                for lst, nm in ((xts, "x"), (hts, "h"), (cts, "c")):
                    lst.append(ina.tile([H, cw * GW], F16,
                                        name=f"{nm}{ci}"))
            def cbounds(ci):
                cs, cw = ICHUNKS[ci]
                return cs * GW, (cs + cw) * GW
            for ci in range(len(ICHUNKS)):
                c0, c1 = cbounds(ci)
                nc.sync.dma_start(xts[ci][:], xt[:, c0:c1])
                nc.sync.dma_start(hts[ci][:], ht[:, c0:c1])
                if ci == 0:
                    wx_sb = const.tile([H, 4 * H], F16)
                    nc.sync.dma_start(wx_sb[:], wxt)
                    wh_sb = const.tile([H, 4 * H], F16)
                    nc.sync.dma_start(wh_sb[:], wht)
                    if has_bias:
                        b_sb = const.tile([H, 4], F32)
                        nc.sync.dma_start(b_sb[:], bias)
                else:
                    c0p, c1p = cbounds(ci - 1)
                    nc.sync.dma_start(cts[ci - 1][:], ct[:, c0p:c1p])
            c0p, c1p = cbounds(len(ICHUNKS) - 1)
            nc.sync.dma_start(cts[-1][:], ct[:, c0p:c1p])

            # ACT table preload (sigmoid set includes tanh) overlaps DMA fill
            dummy = const.tile([H, 8], F32)
            nc.vector.memset(dummy[:], 0.0)
            dummy2 = const.tile([H, 8], F32)
            nc.scalar.activation(dummy2[:], dummy[:], AF.Sigmoid)

            def in_slice(tiles, g, w):
                c0 = g * GW
                for ci, (cs, cw) in enumerate(ICHUNKS):
                    if c0 + w <= (cs + cw) * GW:
                        return tiles[ci][:, c0 - cs * GW:c0 - cs * GW + w]
                raise AssertionError("slice straddles input chunks")

            # HAM warmup on a junk tile while DMAs stream
            junk = const.tile([H, GW], F16)
            nc.vector.memset(junk[:], 0.0)
            warm = qp.tile([H, 2048], F32, name="warm", tag="quad")
            for _ in range(5):
                nc.tensor.matmul(warm[:, 0:GW], junk[:, 0:H], junk[:],
                                 start=True, stop=True)

            NP = NG // 2  # pairs
            sig2s = {}

            # pair -> (chunk_start_group, chunk_width, local_offset, is_last)
            pair_chunk = {}
            for cs, cw in OCHUNKS:
                for g in range(cs, cs + cw, 2):
                    pair_chunk[g // 2] = (cs, cw * GW, (g - cs) * GW,
                                          g + 2 == cs + cw)

            def emit_tanh_h2(Pa):
                """tanh + h' for pairs (Pa, Pa+1) in one ACT pass."""
                Pb = Pa + 1
                cs, cw, lo_a, _ = pair_chunk[Pa]
                cnb, hnb = cn_hn[Pa]
                tcp = tp.tile([H, 2048], BF16, name=f"tc{Pa}", tag="tc")
                nc.scalar.activation(tcp[:], cnb[:, lo_a:lo_a + 4 * GW],
                                     AF.Tanh)
                for j, P in enumerate((Pa, Pb)):
                    lo = pair_chunk[P][2]
                    last = pair_chunk[P][3]
                    sig2 = sig2s.pop(P)
                    o3 = sig2[:].rearrange("p (t x) -> p t x",
                                           t=2)[:, :, 1024:1536]
                    h3 = hnb[:, lo:lo + 2 * GW].rearrange(
                        "p (t x) -> p t x", t=2)
                    t3 = tcp[:, j * 1024:(j + 1) * 1024].rearrange(
                        "p (t x) -> p t x", t=2)
                    nc.vector.tensor_mul(h3, o3, t3)
                    if last:
                        nc.sync.dma_start(hnt[:, cs * GW:cs * GW + cw],
                                          hnb[:])

            def emit_tanh_h(P):
                """tanh + h' + (maybe) hn DMA for pair P (c' already done)."""
                cs, cw, lo, last = pair_chunk[P]
                cnb, hnb = cn_hn[P]
                tcp = tp.tile([H, 1024], BF16, name=f"tc{P}", tag="tc")
                nc.scalar.activation(tcp[:], cnb[:, lo:lo + 2 * GW], AF.Tanh)
                sig2 = sig2s.pop(P)
                o3 = sig2[:].rearrange("p (t x) -> p t x", t=2)[:, :, 1024:1536]
                h3 = hnb[:, lo:lo + 2 * GW].rearrange("p (t x) -> p t x", t=2)
                t3 = tcp[:].rearrange("p (t x) -> p t x", t=2)
                nc.vector.tensor_mul(h3, o3, t3)
                if last:
                    nc.sync.dma_start(hnt[:, cs * GW:cs * GW + cw], hnb[:])

            cn_hn = {}
            cn_buf = hn_buf = None
            for P in range(NP):
                g0 = 2 * P
                cs, cw, lo, last = pair_chunk[P]
                if lo == 0:
                    cn_buf = op.tile([H, cw], F16, name=f"cn{g0}", tag="cn")
                    hn_buf = op.tile([H, cw], F16, name=f"hn{g0}", tag="hn")
                cn_hn[P] = (cn_buf, hn_buf)
                sig2 = sp.tile([H, 4096], BF16, name=f"s{P}", tag="sig")
                sig2s[P] = sig2

                def emit_dve(g_first, ng, tag_sfx):
                    """c'-chain for ng groups starting at g_first (pair P).
                    ig/fc/c' are fp16: bf16 rounding of the large ig/fc
                    terms would dominate the error after cancellation."""
                    w = ng * GW
                    gg = g_first - g0

                    def sl(bank):
                        s = sig2[:].rearrange("p (t x) -> p t x", t=2)
                        s = s[:, gg:gg + ng, bank * GW:(bank + 1) * GW]
                        return s

                    def r3(ap2d):
                        return ap2d.rearrange("p (t x) -> p t x", t=ng)

                    c3 = r3(in_slice(cts, g_first, w))
                    gt = tp.tile([H, w], BF16, name=f"gt{tag_sfx}", tag="gt")
                    nc.vector.tensor_scalar(r3(gt[:]), sl(0 + 3), 2.0, 1.0,
                                            ALU.mult, ALU.subtract)
                    ig = tp.tile([H, w], F16, name=f"ig{tag_sfx}", tag="ig")
                    nc.vector.tensor_mul(r3(ig[:]), sl(0), r3(gt[:]))
                    fc = tp.tile([H, w], F16, name=f"fc{tag_sfx}", tag="fc")
                    nc.vector.tensor_mul(r3(fc[:]), sl(1), c3)
                    lg = lo + gg * GW
                    nc.vector.tensor_add(cn_buf[:, lg:lg + w], ig[:], fc[:])
                    if last and gg + ng == 2:
                        nc.sync.dma_start(
                            cnt[:, cs * GW:cs * GW + cw], cn_buf[:])

                lastP = P == NP - 1
                for gg in range(2):
                    g = g0 + gg
                    xs = in_slice(xts, g, GW)
                    hs = in_slice(hts, g, GW)
                    halves = 2 if (P == 0 and gg == 0 and not has_bias) else 1
                    quad = qp.tile([H, 2048], F32, name=f"q{g}", tag="quad")
                    so = sig2[:, gg * 2048:(gg + 1) * 2048]
                    for half in range(halves):
                        k0, k1 = (0, 4) if halves == 1 else                             (2 * half, 2 * half + 2)
                        for k in range(k0, k1):
                            nc.tensor.matmul(quad[:, k * GW:(k + 1) * GW],
                                             wx_sb[:, k * H:(k + 1) * H], xs,
                                             start=True, stop=False)
                            nc.tensor.matmul(quad[:, k * GW:(k + 1) * GW],
                                             wh_sb[:, k * H:(k + 1) * H], hs,
                                             start=False, stop=True)
                        if has_bias:
                            for k in range(4):
                                nc.scalar.activation(
                                    so[:, k * GW:(k + 1) * GW],
                                    quad[:, k * GW:(k + 1) * GW],
                                    AF.Sigmoid, bias=b_sb[:, k:k + 1])
                        else:
                            c0h, c1h = k0 * GW, k1 * GW
                            nc.scalar.activation(so[:, c0h:c1h],
                                                 quad[:, c0h:c1h], AF.Sigmoid)
                    if lastP or P == 0:
                        # per-group chain: shortens tail (last pair) and
                        # avoids straddling input chunks (first pair)
                        emit_dve(g, 1, f"p{P}g{gg}")
                    if gg == 1:
                        if P in (2, 4, 6):
                            emit_tanh_h2(P - 2)
                        elif P == 7:
                            emit_tanh_h(P - 1)

                if not (lastP or P == 0):
                    emit_dve(g0, 2, f"p{P}")

            # last pair: per-group tanh/h'/hn to shorten the kernel tail
            P = NP - 1
            cs, cw, lo, _ = pair_chunk[P]
            cnb, hnb = cn_hn[P]
            sig2 = sig2s.pop(P)
            for gg in range(2):
                lg = lo + gg * GW
                tcg = tp.tile([H, GW], BF16, name=f"tcz{gg}", tag="tc")
                nc.scalar.activation(tcg[:], cnb[:, lg:lg + GW], AF.Tanh)
                o2 = sig2[:, gg * 2048 + 1024:gg * 2048 + 1536]
                nc.vector.tensor_mul(hnb[:, lg:lg + GW], o2, tcg[:])
                gcol = (cs + gg * (cw // GW - 1)) * GW
                nc.sync.dma_start(hnt[:, gcol:gcol + GW],
                                  hnb[:, lg:lg + GW])
    nc.compile()
    return nc


def _run(inputs, trace=False, tmpdir=None):
    x = np.asarray(inputs["x"], dtype=np.float32)
    h = np.asarray(inputs["h_t"], dtype=np.float32)
    c = np.asarray(inputs["c_t"], dtype=np.float32)
    # gate order [i, f, o, g]; W_g/b_g scaled by 2 for the tanh-via-sigmoid
    wx = np.concatenate([inputs["W_ii"], inputs["W_if"], inputs["W_io"],
                         2.0 * np.asarray(inputs["W_ig"])], axis=0)
    wh = np.concatenate([inputs["W_hi"], inputs["W_hf"], inputs["W_ho"],
                         2.0 * np.asarray(inputs["W_hg"])], axis=0)
    b = np.concatenate([inputs["b_i"], inputs["b_f"], inputs["b_o"],
                        2.0 * np.asarray(inputs["b_g"])], axis=0)
    wxt = np.ascontiguousarray(wx.T).astype(np.float16)
    wht = np.ascontiguousarray(wh.T).astype(np.float16)
    has_bias = bool(np.any(b))

    key = has_bias
    if key not in _CACHE:
        _CACHE[key] = _build(has_bias)
    nc = _CACHE[key]

    x16 = x.astype(np.float16)
    h16 = h.astype(np.float16)
    c16 = c.astype(np.float16)
    in_maps = []
    for i in range(NCORES):
        s = slice(i * BC, (i + 1) * BC)
        m = {
            "xt": np.ascontiguousarray(x16[s].T),
            "ht": np.ascontiguousarray(h16[s].T),
            "ct": np.ascontiguousarray(c16[s].T),
            "wxt": wxt,
            "wht": wht,
        }
        if has_bias:
            m["bias"] = np.ascontiguousarray(
                b.reshape(4, H).T.astype(np.float32))
        in_maps.append(m)

    res = run_bass_kernel_spmd(nc, in_maps, core_ids=list(range(NCORES)),
                               trace=trace, tmpdir=tmpdir)
    h_new = np.empty((NCORES * BC, H), dtype=np.float32)
    c_new = np.empty((NCORES * BC, H), dtype=np.float32)
    for i, r in enumerate(res.results):
        s = slice(i * BC, (i + 1) * BC)
        h_new[s] = r["hnt"].T
        c_new[s] = r["cnt"].T
    return h_new, c_new, res


def kernel(**inputs):
    h_new, c_new, _ = _run(inputs, trace=False)
    return h_new, c_new


# revision 27
# speedup vs baseline: 1.5597x; 1.0047x over previous
"""LSTMCell on 8 Trainium2 NeuronCores, data-parallel over the batch.

Full inputs: x/h_t/c_t [65536,128] f32, 8 gate weight matrices [128,128],
4 biases [128]. Returns (h_new, c_new) as [65536,128] f32 each.

Design (v11, ~61us; fp16 matmul path, transposed layout, no on-device
transposes; steady state is ACT(sigmoid)-bound):
  - Host transposes x/h/c per core to [128 feat, 8192 batch] fp16 and
    pre-concats weights as WxT/WhT [128 in, 512 gate-rows] fp16 in gate
    order [i, f, o, 2*g] (g prescaled by 2 for the tanh-via-sigmoid trick).
    fp16 (not bf16) operands: the bf16 rounding of x/h/W through the gates
    was the dominant error term (1.2e-2); fp16 cuts it ~8x at zero cost
    (PE streams fp16 == bf16: ~216-260ns issue period per N=512 matmul).
  - Per batch group of 512 cols: 8 matmuls (weights stationary) accumulate
    gates^T into a 4-bank PSUM quad [128, 2048] = i|f|o|2g pre-acts.
  - ONE sigmoid per quad -> bf16 SBUF (ACT 16-bit-out runs ~0.9ns/elem;
    bf16 out is fastest of the 16-bit options; f32-out would be 2x faster
    on ACT but forces the DVE chain to 1x mode = net loss). Two groups
    share a sig2 tile [128, 4096] so DVE ops batch per PAIR via 3D APs
    (2-byte dtypes keep the DVE 2x mode, ~0.67ns/elem).
  - DVE per pair: gt=2s-1 [TS], ig=i*gt, fc=f*c, c'=ig+fc, h'=o*tanh(c').
    ig/fc/c'/h' and the c input are fp16: bf16 rounding of the large ig/fc
    terms dominated the error after cancellation in c'.
  - tanh(c') on ACT, delayed one pair (emitted after the next pair's
    sigmoids) so ACT never stalls on the DVE chain; batched across 2 pairs
    mid-pipeline; per-group at the tail to shorten the critical path.
  - DMA: inputs in 5 chunks/tensor (1,1,2,4,8 groups - fast pipeline fill,
    then big descriptors; 4KB-contig descriptors cap the HWDGE ring at
    ~258GB/s so later chunks use 2-8KB lines), x/h ahead of c; outputs in
    8/6/2-group chunks with per-group DMAs at the very end.
"""
import numpy as np
import ml_dtypes
from contextlib import ExitStack

import concourse.bass as bass
import concourse.tile as tile
from concourse import bacc, mybir
from concourse.bass_utils import run_bass_kernel_spmd

F32 = mybir.dt.float32
F16 = mybir.dt.float16
BF16 = mybir.dt.bfloat16
NPBF = ml_dtypes.bfloat16
AF = mybir.ActivationFunctionType
ALU = mybir.AluOpType

NCORES = 8
BC = 8192            # batch rows per core
GW = 512             # batch columns per group (one PSUM bank)
NG = BC // GW        # 16 groups
H = 128              # hidden size
# input chunks in groups: small (fast fill), then growing
ICHUNKS = [(0, 1), (1, 1), (2, 2), (4, 4), (8, 8)]
# output chunks (start group, n groups): big, medium, small tail
OCHUNKS = [(0, 8), (8, 6), (14, 2)]

_CACHE = {}


def _build(has_bias: bool):
    nc = bacc.Bacc("TRN2", target_bir_lowering=False, debug=False)
    xt = nc.dram_tensor("xt", [H, BC], F16, kind="ExternalInput").ap()
    ht = nc.dram_tensor("ht", [H, BC], F16, kind="ExternalInput").ap()
    ct = nc.dram_tensor("ct", [H, BC], F16, kind="ExternalInput").ap()
    wxt = nc.dram_tensor("wxt", [H, 4 * H], F16, kind="ExternalInput").ap()
    wht = nc.dram_tensor("wht", [H, 4 * H], F16, kind="ExternalInput").ap()
    if has_bias:
        bias = nc.dram_tensor("bias", [H, 4], F32, kind="ExternalInput").ap()
    hnt = nc.dram_tensor("hnt", [H, BC], F16, kind="ExternalOutput").ap()
    cnt = nc.dram_tensor("cnt", [H, BC], F16, kind="ExternalOutput").ap()



    with tile.TileContext(nc) as tc:
        with ExitStack() as ctx:
            const = ctx.enter_context(tc.tile_pool(name="const", bufs=1))
            ina = ctx.enter_context(tc.tile_pool(name="ina", bufs=1))
            qp = ctx.enter_context(tc.tile_pool(name="qp", bufs=2, space="PSUM"))
            sp = ctx.enter_context(tc.tile_pool(name="sp", bufs=4))
            tp = ctx.enter_context(tc.tile_pool(name="tp", bufs=2))
            op = ctx.enter_context(tc.tile_pool(name="op", bufs=2))

            # Input tiles in 3 chunks per tensor: small chunk first for fast
            # pipeline fill, then medium/large for DMA efficiency.  x/h
            # chunks issue before c (c is consumed later, by the DVE chain).
            xts, hts, cts = [], [], []
            for ci, (cs, cw) in enumerate(ICHUNKS):
                for lst, nm in ((xts, "x"), (hts, "h"), (cts, "c")):
                    lst.append(ina.tile([H, cw * GW], F16,
                                        name=f"{nm}{ci}"))
            def cbounds(ci):
                cs, cw = ICHUNKS[ci]
                return cs * GW, (cs + cw) * GW
            for ci in range(len(ICHUNKS)):
                c0, c1 = cbounds(ci)
                nc.sync.dma_start(xts[ci][:], xt[:, c0:c1])
                nc.sync.dma_start(hts[ci][:], ht[:, c0:c1])
                if ci == 0:
                    wx_sb = const.tile([H, 4 * H], F16)
                    nc.sync.dma_start(wx_sb[:], wxt)
                    wh_sb = const.tile([H, 4 * H], F16)
                    nc.sync.dma_start(wh_sb[:], wht)
                    if has_bias:
                        b_sb = const.tile([H, 4], F32)
                        nc.sync.dma_start(b_sb[:], bias)
                else:
                    c0p, c1p = cbounds(ci - 1)
                    nc.sync.dma_start(cts[ci - 1][:], ct[:, c0p:c1p])
            c0p, c1p = cbounds(len(ICHUNKS) - 1)
            nc.sync.dma_start(cts[-1][:], ct[:, c0p:c1p])

            # ACT table preload (sigmoid set includes tanh) overlaps DMA fill
            dummy = const.tile([H, 8], F32)
            nc.vector.memset(dummy[:], 0.0)
            dummy2 = const.tile([H, 8], F32)
            nc.scalar.activation(dummy2[:], dummy[:], AF.Sigmoid)

            def in_slice(tiles, g, w):
                c0 = g * GW
                for ci, (cs, cw) in enumerate(ICHUNKS):
                    if c0 + w <= (cs + cw) * GW:
                        return tiles[ci][:, c0 - cs * GW:c0 - cs * GW + w]
                raise AssertionError("slice straddles input chunks")

            # HAM warmup on a junk tile while DMAs stream
            junk = const.tile([H, GW], F16)
            nc.vector.memset(junk[:], 0.0)
            warm = qp.tile([H, 2048], F32, name="warm", tag="quad")
            for _ in range(9):
                nc.tensor.matmul(warm[:, 0:GW], junk[:, 0:H], junk[:],
                                 start=True, stop=True)

            NP = NG // 2  # pairs
            sig2s = {}

            # pair -> (chunk_start_group, chunk_width, local_offset, is_last)
            pair_chunk = {}
            for cs, cw in OCHUNKS:
                for g in range(cs, cs + cw, 2):
                    pair_chunk[g // 2] = (cs, cw * GW, (g - cs) * GW,
                                          g + 2 == cs + cw)

            def emit_tanh_h2(Pa):
                """tanh + h' for pairs (Pa, Pa+1) in one ACT pass."""
                Pb = Pa + 1
                cs, cw, lo_a, _ = pair_chunk[Pa]
                cnb, hnb = cn_hn[Pa]
                tcp = tp.tile([H, 2048], BF16, name=f"tc{Pa}", tag="tc")
                nc.scalar.activation(tcp[:], cnb[:, lo_a:lo_a + 4 * GW],
                                     AF.Tanh)
                for j, P in enumerate((Pa, Pb)):
                    lo = pair_chunk[P][2]
                    last = pair_chunk[P][3]
                    sig2 = sig2s.pop(P)
                    o3 = sig2[:].rearrange("p (t x) -> p t x",
                                           t=2)[:, :, 0:512]
                    h3 = hnb[:, lo:lo + 2 * GW].rearrange(
                        "p (t x) -> p t x", t=2)
                    t3 = tcp[:, j * 1024:(j + 1) * 1024].rearrange(
                        "p (t x) -> p t x", t=2)
                    nc.vector.tensor_mul(h3, o3, t3)
                    if last:
                        nc.sync.dma_start(hnt[:, cs * GW:cs * GW + cw],
                                          hnb[:])

            def emit_tanh_h(P):
                """tanh + h' + (maybe) hn DMA for pair P (c' already done)."""
                cs, cw, lo, last = pair_chunk[P]
                cnb, hnb = cn_hn[P]
                tcp = tp.tile([H, 1024], BF16, name=f"tc{P}", tag="tc")
                nc.scalar.activation(tcp[:], cnb[:, lo:lo + 2 * GW], AF.Tanh)
                sig2 = sig2s.pop(P)
                o3 = sig2[:].rearrange("p (t x) -> p t x", t=2)[:, :, 0:512]
                h3 = hnb[:, lo:lo + 2 * GW].rearrange("p (t x) -> p t x", t=2)
                t3 = tcp[:].rearrange("p (t x) -> p t x", t=2)
                nc.vector.tensor_mul(h3, o3, t3)
                if last:
                    nc.sync.dma_start(hnt[:, cs * GW:cs * GW + cw], hnb[:])

            cn_hn = {}
            cn_buf = hn_buf = None
            for P in range(NP):
                g0 = 2 * P
                cs, cw, lo, last = pair_chunk[P]
                if lo == 0:
                    cn_buf = op.tile([H, cw], F16, name=f"cn{g0}", tag="cn")
                    hn_buf = op.tile([H, cw], F16, name=f"hn{g0}", tag="hn")
                cn_hn[P] = (cn_buf, hn_buf)
                sig2 = sp.tile([H, 4096], BF16, name=f"s{P}", tag="sig")
                sig2s[P] = sig2

                def emit_dve(g_first, ng, tag_sfx):
                    """c'-chain for ng groups starting at g_first (pair P).
                    ig/fc/c' are fp16: bf16 rounding of the large ig/fc
                    terms would dominate the error after cancellation."""
                    w = ng * GW
                    gg = g_first - g0

                    def sl(bank):
                        s = sig2[:].rearrange("p (t x) -> p t x", t=2)
                        s = s[:, gg:gg + ng, bank * GW:(bank + 1) * GW]
                        return s

                    def r3(ap2d):
                        return ap2d.rearrange("p (t x) -> p t x", t=ng)

                    c3 = r3(in_slice(cts, g_first, w))
                    gt = tp.tile([H, w], BF16, name=f"gt{tag_sfx}", tag="gt")
                    nc.vector.tensor_scalar(r3(gt[:]), sl(0 + 3), 2.0, 1.0,
                                            ALU.mult, ALU.subtract)
                    ig = tp.tile([H, w], F16, name=f"ig{tag_sfx}", tag="ig")
                    nc.vector.tensor_mul(r3(ig[:]), sl(1), r3(gt[:]))
                    fc = tp.tile([H, w], F16, name=f"fc{tag_sfx}", tag="fc")
                    nc.vector.tensor_mul(r3(fc[:]), sl(2), c3)
                    lg = lo + gg * GW
                    nc.vector.tensor_add(cn_buf[:, lg:lg + w], ig[:], fc[:])
                    if last and gg + ng == 2:
                        nc.sync.dma_start(
                            cnt[:, cs * GW:cs * GW + cw], cn_buf[:])

                lastP = P == NP - 1
                for gg in range(2):
                    g = g0 + gg
                    xs = in_slice(xts, g, GW)
                    hs = in_slice(hts, g, GW)
                    split = (lastP or P == 0) and not has_bias
                    quad = qp.tile([H, 2048], F32, name=f"q{g}", tag="quad")
                    so = sig2[:, gg * 2048:(gg + 1) * 2048]
                    for k in ([1, 2, 3, 0] if split else range(4)):
                        nc.tensor.matmul(quad[:, k * GW:(k + 1) * GW],
                                         wx_sb[:, k * H:(k + 1) * H], xs,
                                         start=True, stop=False)
                        nc.tensor.matmul(quad[:, k * GW:(k + 1) * GW],
                                         wh_sb[:, k * H:(k + 1) * H], hs,
                                         start=False, stop=True)
                    if has_bias:
                        for k in range(4):
                            nc.scalar.activation(
                                so[:, k * GW:(k + 1) * GW],
                                quad[:, k * GW:(k + 1) * GW],
                                AF.Sigmoid, bias=b_sb[:, k:k + 1])
                    elif split:
                        # i/f/s banks first: unblocks the DVE chain; the
                        # o bank (only needed by h') trails
                        nc.scalar.activation(so[:, GW:], quad[:, GW:],
                                             AF.Sigmoid)
                        nc.scalar.activation(so[:, 0:GW], quad[:, 0:GW],
                                             AF.Sigmoid)
                    else:
                        nc.scalar.activation(so, quad[:], AF.Sigmoid)
                    if lastP or P == 0:
                        # per-group chain: shortens tail (last pair) and
                        # avoids straddling input chunks (first pair)
                        emit_dve(g, 1, f"p{P}g{gg}")
                    if gg == 1:
                        if P in (2, 4, 6):
                            emit_tanh_h2(P - 2)
                        elif P == 7:
                            emit_tanh_h(P - 1)

                if not (lastP or P == 0):
                    emit_dve(g0, 2, f"p{P}")

            # last pair: per-group tanh/h'/hn to shorten the kernel tail
            P = NP - 1
            cs, cw, lo, _ = pair_chunk[P]
            cnb, hnb = cn_hn[P]
            sig2 = sig2s.pop(P)
            for gg in range(2):
                lg = lo + gg * GW
                tcg = tp.tile([H, GW], BF16, name=f"tcz{gg}", tag="tc")
                nc.scalar.activation(tcg[:], cnb[:, lg:lg + GW], AF.Tanh)
                o2 = sig2[:, gg * 2048:gg * 2048 + 512]
                nc.vector.tensor_mul(hnb[:, lg:lg + GW], o2, tcg[:])
                gcol = (cs + gg * (cw // GW - 1)) * GW
                nc.sync.dma_start(hnt[:, gcol:gcol + GW],
                                  hnb[:, lg:lg + GW])
    nc.compile()
    return nc


def _run(inputs, trace=False, tmpdir=None):
    x = np.asarray(inputs["x"], dtype=np.float32)
    h = np.asarray(inputs["h_t"], dtype=np.float32)
    c = np.asarray(inputs["c_t"], dtype=np.float32)
    # gate order [i, f, o, g]; W_g/b_g scaled by 2 for the tanh-via-sigmoid
    wx = np.concatenate([inputs["W_io"], inputs["W_ii"], inputs["W_if"],
                         2.0 * np.asarray(inputs["W_ig"])], axis=0)
    wh = np.concatenate([inputs["W_ho"], inputs["W_hi"], inputs["W_hf"],
                         2.0 * np.asarray(inputs["W_hg"])], axis=0)
    b = np.concatenate([inputs["b_o"], inputs["b_i"], inputs["b_f"],
                        2.0 * np.asarray(inputs["b_g"])], axis=0)
    wxt = np.ascontiguousarray(wx.T).astype(np.float16)
    wht = np.ascontiguousarray(wh.T).astype(np.float16)
    has_bias = bool(np.any(b))

    key = has_bias
    if key not in _CACHE:
        _CACHE[key] = _build(has_bias)
    nc = _CACHE[key]

    x16 = x.astype(np.float16)
    h16 = h.astype(np.float16)
    c16 = c.astype(np.float16)
    in_maps = []
    for i in range(NCORES):
        s = slice(i * BC, (i + 1) * BC)
        m = {
            "xt": np.ascontiguousarray(x16[s].T),
            "ht": np.ascontiguousarray(h16[s].T),
            "ct": np.ascontiguousarray(c16[s].T),
            "wxt": wxt,
            "wht": wht,
        }
        if has_bias:
            m["bias"] = np.ascontiguousarray(
                b.reshape(4, H).T.astype(np.float32))
        in_maps.append(m)

    res = run_bass_kernel_spmd(nc, in_maps, core_ids=list(range(NCORES)),
                               trace=trace, tmpdir=tmpdir)
    h_new = np.empty((NCORES * BC, H), dtype=np.float32)
    c_new = np.empty((NCORES * BC, H), dtype=np.float32)
    for i, r in enumerate(res.results):
        s = slice(i * BC, (i + 1) * BC)
        h_new[s] = r["hnt"].T
        c_new[s] = r["cnt"].T
    return h_new, c_new, res


def kernel(**inputs):
    h_new, c_new, _ = _run(inputs, trace=False)
    return h_new, c_new


# revision 28
# speedup vs baseline: 1.5766x; 1.0108x over previous
"""LSTMCell on 8 Trainium2 NeuronCores, data-parallel over the batch.

Full inputs: x/h_t/c_t [65536,128] f32, 8 gate weight matrices [128,128],
4 biases [128]. Returns (h_new, c_new) as [65536,128] f32 each.

Design (v11, ~61us; fp16 matmul path, transposed layout, no on-device
transposes; steady state is ACT(sigmoid)-bound):
  - Host transposes x/h/c per core to [128 feat, 8192 batch] fp16 and
    pre-concats weights as WxT/WhT [128 in, 512 gate-rows] fp16 in gate
    order [i, f, o, 2*g] (g prescaled by 2 for the tanh-via-sigmoid trick).
    fp16 (not bf16) operands: the bf16 rounding of x/h/W through the gates
    was the dominant error term (1.2e-2); fp16 cuts it ~8x at zero cost
    (PE streams fp16 == bf16: ~216-260ns issue period per N=512 matmul).
  - Per batch group of 512 cols: 8 matmuls (weights stationary) accumulate
    gates^T into a 4-bank PSUM quad [128, 2048] = i|f|o|2g pre-acts.
  - ONE sigmoid per quad -> bf16 SBUF (ACT 16-bit-out runs ~0.9ns/elem;
    bf16 out is fastest of the 16-bit options; f32-out would be 2x faster
    on ACT but forces the DVE chain to 1x mode = net loss). Two groups
    share a sig2 tile [128, 4096] so DVE ops batch per PAIR via 3D APs
    (2-byte dtypes keep the DVE 2x mode, ~0.67ns/elem).
  - DVE per pair: gt=2s-1 [TS], ig=i*gt, fc=f*c, c'=ig+fc, h'=o*tanh(c').
    ig/fc/c'/h' and the c input are fp16: bf16 rounding of the large ig/fc
    terms dominated the error after cancellation in c'.
  - tanh(c') on ACT, delayed one pair (emitted after the next pair's
    sigmoids) so ACT never stalls on the DVE chain; batched across 2 pairs
    mid-pipeline; per-group at the tail to shorten the critical path.
  - DMA: inputs in 5 chunks/tensor (1,1,2,4,8 groups - fast pipeline fill,
    then big descriptors; 4KB-contig descriptors cap the HWDGE ring at
    ~258GB/s so later chunks use 2-8KB lines), x/h ahead of c; outputs in
    8/6/2-group chunks with per-group DMAs at the very end.
"""
import numpy as np
import ml_dtypes
from contextlib import ExitStack

import concourse.bass as bass
import concourse.tile as tile
from concourse import bacc, mybir
from concourse.bass_utils import run_bass_kernel_spmd

F32 = mybir.dt.float32
F16 = mybir.dt.float16
BF16 = mybir.dt.bfloat16
NPBF = ml_dtypes.bfloat16
AF = mybir.ActivationFunctionType
ALU = mybir.AluOpType

NCORES = 8
BC = 8192            # batch rows per core
GW = 512             # batch columns per group (one PSUM bank)
NG = BC // GW        # 16 groups
H = 128              # hidden size
# input chunks in groups: small (fast fill), then growing
ICHUNKS = [(0, 1), (1, 1), (2, 2), (4, 4), (8, 8)]
# output chunks (start group, n groups): big, medium, small tail
OCHUNKS = [(0, 8), (8, 4), (12, 2), (14, 2)]

_CACHE = {}


def _build(has_bias: bool):
    nc = bacc.Bacc("TRN2", target_bir_lowering=False, debug=False)
    xt = nc.dram_tensor("xt", [H, BC], F16, kind="ExternalInput").ap()
    ht = nc.dram_tensor("ht", [H, BC], F16, kind="ExternalInput").ap()
    ct = nc.dram_tensor("ct", [H, BC], F16, kind="ExternalInput").ap()
    wxt = nc.dram_tensor("wxt", [H, 4 * H], F16, kind="ExternalInput").ap()
    wht = nc.dram_tensor("wht", [H, 4 * H], F16, kind="ExternalInput").ap()
    if has_bias:
        bias = nc.dram_tensor("bias", [H, 4], F32, kind="ExternalInput").ap()
    hnt = nc.dram_tensor("hnt", [H, BC], F16, kind="ExternalOutput").ap()
    cnt = nc.dram_tensor("cnt", [H, BC], F16, kind="ExternalOutput").ap()



    with tile.TileContext(nc) as tc:
        with ExitStack() as ctx:
            const = ctx.enter_context(tc.tile_pool(name="const", bufs=1))
            ina = ctx.enter_context(tc.tile_pool(name="ina", bufs=1))
            qp = ctx.enter_context(tc.tile_pool(name="qp", bufs=2, space="PSUM"))
            sp = ctx.enter_context(tc.tile_pool(name="sp", bufs=4))
            tp = ctx.enter_context(tc.tile_pool(name="tp", bufs=2))
            op = ctx.enter_context(tc.tile_pool(name="op", bufs=2))

            # Input tiles in 3 chunks per tensor: small chunk first for fast
            # pipeline fill, then medium/large for DMA efficiency.  x/h
            # chunks issue before c (c is consumed later, by the DVE chain).
            xts, hts, cts = [], [], []
            for ci, (cs, cw) in enumerate(ICHUNKS):
                for lst, nm in ((xts, "x"), (hts, "h"), (cts, "c")):
                    lst.append(ina.tile([H, cw * GW], F16,
                                        name=f"{nm}{ci}"))
            def cbounds(ci):
                cs, cw = ICHUNKS[ci]
                return cs * GW, (cs + cw) * GW
            for ci in range(len(ICHUNKS)):
                c0, c1 = cbounds(ci)
                nc.sync.dma_start(xts[ci][:], xt[:, c0:c1])
                nc.sync.dma_start(hts[ci][:], ht[:, c0:c1])
                if ci == 0:
                    wx_sb = const.tile([H, 4 * H], F16)
                    nc.sync.dma_start(wx_sb[:], wxt)
                    wh_sb = const.tile([H, 4 * H], F16)
                    nc.sync.dma_start(wh_sb[:], wht)
                    if has_bias:
                        b_sb = const.tile([H, 4], F32)
                        nc.sync.dma_start(b_sb[:], bias)
                else:
                    c0p, c1p = cbounds(ci - 1)
                    nc.sync.dma_start(cts[ci - 1][:], ct[:, c0p:c1p])
            c0p, c1p = cbounds(len(ICHUNKS) - 1)
            nc.sync.dma_start(cts[-1][:], ct[:, c0p:c1p])

            # ACT table preload (sigmoid set includes tanh) overlaps DMA fill
            dummy = const.tile([H, 8], F32)
            nc.vector.memset(dummy[:], 0.0)
            dummy2 = const.tile([H, 8], F32)
            nc.scalar.activation(dummy2[:], dummy[:], AF.Sigmoid)

            def in_slice(tiles, g, w):
                c0 = g * GW
                for ci, (cs, cw) in enumerate(ICHUNKS):
                    if c0 + w <= (cs + cw) * GW:
                        return tiles[ci][:, c0 - cs * GW:c0 - cs * GW + w]
                raise AssertionError("slice straddles input chunks")

            # HAM warmup on a junk tile while DMAs stream
            junk = const.tile([H, GW], F16)
            nc.vector.memset(junk[:], 0.0)
            warm = qp.tile([H, 2048], F32, name="warm", tag="quad")
            for _ in range(9):
                nc.tensor.matmul(warm[:, 0:GW], junk[:, 0:H], junk[:],
                                 start=True, stop=True)

            NP = NG // 2  # pairs
            sig2s = {}

            # pair -> (chunk_start_group, chunk_width, local_offset, is_last)
            pair_chunk = {}
            for cs, cw in OCHUNKS:
                for g in range(cs, cs + cw, 2):
                    pair_chunk[g // 2] = (cs, cw * GW, (g - cs) * GW,
                                          g + 2 == cs + cw)

            def emit_tanh_h2(Pa):
                """tanh + h' for pairs (Pa, Pa+1) in one ACT pass."""
                Pb = Pa + 1
                cs, cw, lo_a, _ = pair_chunk[Pa]
                cnb, hnb = cn_hn[Pa]
                tcp = tp.tile([H, 2048], BF16, name=f"tc{Pa}", tag="tc")
                nc.scalar.activation(tcp[:], cnb[:, lo_a:lo_a + 4 * GW],
                                     AF.Tanh)
                for j, P in enumerate((Pa, Pb)):
                    lo = pair_chunk[P][2]
                    last = pair_chunk[P][3]
                    sig2 = sig2s.pop(P)
                    o3 = sig2[:].rearrange("p (t x) -> p t x",
                                           t=2)[:, :, 0:512]
                    h3 = hnb[:, lo:lo + 2 * GW].rearrange(
                        "p (t x) -> p t x", t=2)
                    t3 = tcp[:, j * 1024:(j + 1) * 1024].rearrange(
                        "p (t x) -> p t x", t=2)
                    nc.vector.tensor_mul(h3, o3, t3)
                    if last:
                        nc.sync.dma_start(hnt[:, cs * GW:cs * GW + cw],
                                          hnb[:])

            def emit_tanh_h(P):
                """tanh + h' + (maybe) hn DMA for pair P (c' already done)."""
                cs, cw, lo, last = pair_chunk[P]
                cnb, hnb = cn_hn[P]
                tcp = tp.tile([H, 1024], BF16, name=f"tc{P}", tag="tc")
                nc.scalar.activation(tcp[:], cnb[:, lo:lo + 2 * GW], AF.Tanh)
                sig2 = sig2s.pop(P)
                o3 = sig2[:].rearrange("p (t x) -> p t x", t=2)[:, :, 0:512]
                h3 = hnb[:, lo:lo + 2 * GW].rearrange("p (t x) -> p t x", t=2)
                t3 = tcp[:].rearrange("p (t x) -> p t x", t=2)
                nc.vector.tensor_mul(h3, o3, t3)
                if last:
                    nc.sync.dma_start(hnt[:, cs * GW:cs * GW + cw], hnb[:])

            cn_hn = {}
            cn_buf = hn_buf = None
            for P in range(NP):
                g0 = 2 * P
                cs, cw, lo, last = pair_chunk[P]
                if lo == 0:
                    cn_buf = op.tile([H, cw], F16, name=f"cn{g0}", tag="cn")
                    hn_buf = op.tile([H, cw], F16, name=f"hn{g0}", tag="hn")
                cn_hn[P] = (cn_buf, hn_buf)
                sig2 = sp.tile([H, 4096], BF16, name=f"s{P}", tag="sig")
                sig2s[P] = sig2

                def emit_dve(g_first, ng, tag_sfx):
                    """c'-chain for ng groups starting at g_first (pair P).
                    ig/fc/c' are fp16: bf16 rounding of the large ig/fc
                    terms would dominate the error after cancellation."""
                    w = ng * GW
                    gg = g_first - g0

                    def sl(bank):
                        s = sig2[:].rearrange("p (t x) -> p t x", t=2)
                        s = s[:, gg:gg + ng, bank * GW:(bank + 1) * GW]
                        return s

                    def r3(ap2d):
                        return ap2d.rearrange("p (t x) -> p t x", t=ng)

                    c3 = r3(in_slice(cts, g_first, w))
                    gt = tp.tile([H, w], BF16, name=f"gt{tag_sfx}", tag="gt")
                    nc.vector.tensor_scalar(r3(gt[:]), sl(0 + 3), 2.0, 1.0,
                                            ALU.mult, ALU.subtract)
                    ig = tp.tile([H, w], F16, name=f"ig{tag_sfx}", tag="ig")
                    nc.vector.tensor_mul(r3(ig[:]), sl(1), r3(gt[:]))
                    fc = tp.tile([H, w], F16, name=f"fc{tag_sfx}", tag="fc")
                    nc.vector.tensor_mul(r3(fc[:]), sl(2), c3)
                    lg = lo + gg * GW
                    nc.vector.tensor_add(cn_buf[:, lg:lg + w], ig[:], fc[:])
                    if last and gg + ng == 2:
                        nc.sync.dma_start(
                            cnt[:, cs * GW:cs * GW + cw], cn_buf[:])

                lastP = P == NP - 1
                for gg in range(2):
                    g = g0 + gg
                    xs = in_slice(xts, g, GW)
                    hs = in_slice(hts, g, GW)
                    split = (lastP or P == 0) and not has_bias
                    quad = qp.tile([H, 2048], F32, name=f"q{g}", tag="quad")
                    so = sig2[:, gg * 2048:(gg + 1) * 2048]
                    for k in ([1, 2, 3, 0] if split else range(4)):
                        nc.tensor.matmul(quad[:, k * GW:(k + 1) * GW],
                                         wx_sb[:, k * H:(k + 1) * H], xs,
                                         start=True, stop=False)
                        nc.tensor.matmul(quad[:, k * GW:(k + 1) * GW],
                                         wh_sb[:, k * H:(k + 1) * H], hs,
                                         start=False, stop=True)
                    if has_bias:
                        for k in range(4):
                            nc.scalar.activation(
                                so[:, k * GW:(k + 1) * GW],
                                quad[:, k * GW:(k + 1) * GW],
                                AF.Sigmoid, bias=b_sb[:, k:k + 1])
                    elif split:
                        # i/f/s banks first: unblocks the DVE chain; the
                        # o bank (only needed by h') trails
                        nc.scalar.activation(so[:, GW:], quad[:, GW:],
                                             AF.Sigmoid)
                        nc.scalar.activation(so[:, 0:GW], quad[:, 0:GW],
                                             AF.Sigmoid)
                    else:
                        nc.scalar.activation(so, quad[:], AF.Sigmoid)
                    if lastP or P == 0:
                        # per-group chain: shortens tail (last pair) and
                        # avoids straddling input chunks (first pair)
                        emit_dve(g, 1, f"p{P}g{gg}")
                    if gg == 1:
                        if P in (2, 4, 6):
                            emit_tanh_h2(P - 2)
                        elif P == 7:
                            emit_tanh_h(P - 1)

                if not (lastP or P == 0):
                    emit_dve(g0, 2, f"p{P}")

            # last pair: per-group tanh/h'/hn to shorten the kernel tail
            P = NP - 1
            cs, cw, lo, _ = pair_chunk[P]
            cnb, hnb = cn_hn[P]
            sig2 = sig2s.pop(P)
            for gg in range(2):
                lg = lo + gg * GW
                tcg = tp.tile([H, GW], BF16, name=f"tcz{gg}", tag="tc")
                nc.scalar.activation(tcg[:], cnb[:, lg:lg + GW], AF.Tanh)
                o2 = sig2[:, gg * 2048:gg * 2048 + 512]
                nc.vector.tensor_mul(hnb[:, lg:lg + GW], o2, tcg[:])
                gcol = (cs + gg * (cw // GW - 1)) * GW
                nc.sync.dma_start(hnt[:, gcol:gcol + GW],
                                  hnb[:, lg:lg + GW])
    nc.compile()
    return nc


def _run(inputs, trace=False, tmpdir=None):
    x = np.asarray(inputs["x"], dtype=np.float32)
    h = np.asarray(inputs["h_t"], dtype=np.float32)
    c = np.asarray(inputs["c_t"], dtype=np.float32)
    # gate order [i, f, o, g]; W_g/b_g scaled by 2 for the tanh-via-sigmoid
    wx = np.concatenate([inputs["W_io"], inputs["W_ii"], inputs["W_if"],
                         2.0 * np.asarray(inputs["W_ig"])], axis=0)
    wh = np.concatenate([inputs["W_ho"], inputs["W_hi"], inputs["W_hf"],
                         2.0 * np.asarray(inputs["W_hg"])], axis=0)
    b = np.concatenate([inputs["b_o"], inputs["b_i"], inputs["b_f"],
                        2.0 * np.asarray(inputs["b_g"])], axis=0)
    wxt = np.ascontiguousarray(wx.T).astype(np.float16)
    wht = np.ascontiguousarray(wh.T).astype(np.float16)
    has_bias = bool(np.any(b))

    key = has_bias
    if key not in _CACHE:
        _CACHE[key] = _build(has_bias)
    nc = _CACHE[key]

    x16 = x.astype(np.float16)
    h16 = h.astype(np.float16)
    c16 = c.astype(np.float16)
    in_maps = []
    for i in range(NCORES):
        s = slice(i * BC, (i + 1) * BC)
        m = {
            "xt": np.ascontiguousarray(x16[s].T),
            "ht": np.ascontiguousarray(h16[s].T),
            "ct": np.ascontiguousarray(c16[s].T),
            "wxt": wxt,
            "wht": wht,
        }
        if has_bias:
            m["bias"] = np.ascontiguousarray(
                b.reshape(4, H).T.astype(np.float32))
        in_maps.append(m)

    res = run_bass_kernel_spmd(nc, in_maps, core_ids=list(range(NCORES)),
                               trace=trace, tmpdir=tmpdir)
    h_new = np.empty((NCORES * BC, H), dtype=np.float32)
    c_new = np.empty((NCORES * BC, H), dtype=np.float32)
    for i, r in enumerate(res.results):
        s = slice(i * BC, (i + 1) * BC)
        h_new[s] = r["hnt"].T
        c_new[s] = r["cnt"].T
    return h_new, c_new, res


def kernel(**inputs):
    h_new, c_new, _ = _run(inputs, trace=False)
    return h_new, c_new
